# revision 28
# baseline (speedup 1.0000x reference)
"""AssignmentSimilarityNet GNN message-passing kernel for 8 Trainium2
NeuronCores.

Sharding: track (A) dimension split across 8 cores (32 tracks each).
Edge tensors, track embeds and messages-to-A stay local; messages-to-B
(sum over A) are all-reduced each step; MLP weights replicated.

Schedule (per step, steady state) — built around the ~23us serial
latency of one 8-core AllReduce (collectives cannot be pipelined in
the CC engine, so exactly one AR per step, fully covered):

  cover section (AR(k-1) in flight):
    - te(k) node update (local, from msga(k-1)); Tb = W1a@te + be1
    - S1 prologue: first two chunks' fixed+upd K-tile matmuls into PSUM
    - classifier chunks 0..5 of step k-1, software-pipelined on the PE
      (wc1 of chunk c+1 issues before the M=1 wc2 of chunk c); logits
      land in (1,512) PSUM rows, staged to SBUF (vector/scalar
      alternating), one small DMA per chunk
  AR-dependent section:
    - fetch msgb (split across sync+scalar queues), ce(k) node update
      with classifier chunks 6..8 interleaved into the PE queue so the
      in-order queue has work during the scalar hops; C = W1b @ ce(k)
    - per chunk: t = (p1 + Tb_col) + C via two scalar_tensor_tensor ops
      (vector), relu in place (scalar), we2 matmul, u = relu(p2+be2)
      (scalar); remaining S1 and classifier chunks interleaved
    - msgb via contiguous pair adds + two accumulator chains (vector);
      msga via 4 contiguous strided-view reduces off the critical path;
      stage DMA, trigger AR(k) (gpsimd queue holds only the triggers)
  The gpsimd/Pool engine is useless for bulk elementwise work (~9us for
  a 512-wide relu) and cannot touch PSUM; it only issues the collective
  triggers so no compute queue ever blocks on an AR.

The 8th AR is skipped (last node update is dead).  b_c2 is added on the
host.  All compute is fp32/fp32r: bf16 anywhere in the recurrent or
message path overshoots the 2e-2 error budget (measured 2.6e-2 for a
bf16 msgb alone; the net amplifies per-op rounding ~100x over 8 steps).
"""
import numpy as np

A = 256          # tracks
B = 256          # current detections
AL = A // 8      # tracks per core (32)
REID = 512
D = 128          # ND == ED
STEPS = 8
NP = AL * B      # pairs per core (8192)
CH = 512         # pair chunk (2 a-groups x 256 b)
NCH = NP // CH   # 16

_BUILD_CACHE = {}


def _build():
    if "nc" in _BUILD_CACHE:
        return _BUILD_CACHE["nc"]
    import concourse.bacc as bacc
    import concourse.mybir as mybir
    import concourse.tile as tile

    F32 = mybir.dt.float32
    F32R = mybir.dt.float32r
    AF = mybir.ActivationFunctionType
    ALU = mybir.AluOpType

    nc = bacc.Bacc(None, target_bir_lowering=False)

    def din(name, shape):
        return nc.dram_tensor(name, shape, F32, kind="ExternalInput")

    tfT = din("tfT", [REID, AL])
    trkf = din("trkf", [AL, REID])
    cfT = din("cfT", [REID, B])
    curf = din("curf", [B, REID])
    trkg = din("trkg", [AL, 5])
    curg = din("curg", [B, 5])
    wlinT = din("wlinT", [REID, D])
    blin = din("blin", [D, 1])
    wein1T = din("wein1T", [6, D])
    bein1 = din("bein1", [D, 1])
    wein2T = din("wein2T", [D, D])
    bein2 = din("bein2", [D, 1])
    we1T = din("we1T", [4 * D, D])
    be1 = din("be1", [D, 1])
    we2T = din("we2T", [D, D])
    be2 = din("be2", [D, 1])
    wn1T = din("wn1T", [2 * D, D])
    bn1 = din("bn1", [D, 1])
    wn2T = din("wn2T", [D, D])
    bn2 = din("bn2", [D, 1])
    wc1T = din("wc1T", [D, D])
    bc1 = din("bc1", [D, 1])
    wc2c = din("wc2c", [D, 1])
    out = nc.dram_tensor("out", [STEPS, NP], F32, kind="ExternalOutput")

    with tile.TileContext(nc) as tc:
        with (
            tc.tile_pool(name="const", bufs=1) as cp,
            tc.tile_pool(name="state", bufs=1) as st,
            tc.tile_pool(name="work", bufs=1) as wk,
            tc.tile_pool(name="p1", bufs=2, space="PSUM") as pp1,
            tc.tile_pool(name="p2", bufs=2, space="PSUM") as pp2,
            tc.tile_pool(name="p3", bufs=2, space="PSUM") as pp3,
            tc.tile_pool(name="plg", bufs=2, space="PSUM") as plgp,
            tc.tile_pool(name="dram", bufs=1, space="DRAM") as dr,
        ):
            # ---------------- feature loads ----------------
            tf_t = st.tile([128, 4 * AL], F32R)       # 4 K-tiles of (128, 32)
            cf_t = st.tile([128, 4 * B], F32R)        # 4 K-tiles of (128, 256)
            tf_s = wk.tile([128, 4 * AL], F32, tag="wstage", bufs=2)
            cf_s = wk.tile([128, 4 * B], F32, tag="wstage", bufs=2)
            for j in range(4):
                nc.sync.dma_start(tf_s[:, AL * j:AL * (j + 1)],
                                  tfT[128 * j:128 * (j + 1), :])
                nc.sync.dma_start(cf_s[:, B * j:B * (j + 1)],
                                  cfT[128 * j:128 * (j + 1), :])
            for j in range(4):
                nc.vector.tensor_copy(tf_t[:, AL * j:AL * (j + 1)],
                                      tf_s[:, AL * j:AL * (j + 1)])
                nc.vector.tensor_copy(cf_t[:, B * j:B * (j + 1)],
                                      cf_s[:, B * j:B * (j + 1)])
            trkf_t = wk.tile([AL, REID], F32)
            nc.scalar.dma_start(trkf_t[:], trkf[:])
            curf_t0 = wk.tile([128, REID], F32)
            curf_t1 = wk.tile([128, REID], F32)
            nc.scalar.dma_start(curf_t0[:], curf[0:128, :])
            nc.scalar.dma_start(curf_t1[:], curf[128:256, :])
            trkg_t = wk.tile([AL, 5], F32)
            nc.scalar.dma_start(trkg_t[:], trkg[:])
            curg_t0 = wk.tile([128, 5], F32)
            curg_t1 = wk.tile([128, 5], F32)
            nc.scalar.dma_start(curg_t0[:], curg[0:128, :])
            nc.scalar.dma_start(curg_t1[:], curg[128:256, :])

            # ---------------- reid norms ----------------
            sq_t = wk.tile([AL, REID], F32, tag="sq", bufs=1)
            nc.vector.tensor_mul(sq_t[:], trkf_t[:], trkf_t[:])
            sst = wk.tile([AL, 1], F32)
            nc.vector.tensor_reduce(sst[:], sq_t[:], mybir.AxisListType.X, ALU.add)
            rt = wk.tile([AL, 1], F32)
            nc.vector.reciprocal(rt[:], sst[:])
            inv_t = wk.tile([AL, 1], F32)
            nc.scalar.activation(inv_t[:], rt[:], AF.Sqrt)

            invc = []
            for i, ct in enumerate((curf_t0, curf_t1)):
                sq_c = wk.tile([128, REID], F32, name=f"sq_c{i}", tag="sq", bufs=1)
                nc.vector.tensor_mul(sq_c[:], ct[:], ct[:])
                ssc = wk.tile([128, 1], F32, name=f"ssc{i}")
                nc.vector.tensor_reduce(ssc[:], sq_c[:], mybir.AxisListType.X,
                                        ALU.add)
                rc = wk.tile([128, 1], F32, name=f"rc{i}")
                nc.vector.reciprocal(rc[:], ssc[:])
                ic = wk.tile([128, 1], F32, name=f"ic{i}")
                nc.scalar.activation(ic[:], rc[:], AF.Sqrt)
                invc.append(ic)

            # ---------------- current-side geometry -> bcast rows ----------
            # rows of cstage: 0 xb, 1 yb, 2 hb, 3 ln hb, 4 ln wb, 5 tb, 6 invc
            cstage = dr.tile([7, B], F32)
            for i, gt in enumerate((curg_t0, curg_t1)):
                half = slice(128 * i, 128 * (i + 1))
                cg = wk.tile([128, 7], F32, name=f"cg{i}")
                nc.vector.tensor_add(cg[:, 0:1], gt[:, 0:1], gt[:, 2:3])
                nc.vector.tensor_scalar_mul(cg[:, 0:1], cg[:, 0:1], 0.5)
                nc.vector.tensor_add(cg[:, 1:2], gt[:, 1:2], gt[:, 3:4])
                nc.vector.tensor_scalar_mul(cg[:, 1:2], cg[:, 1:2], 0.5)
                nc.vector.tensor_sub(cg[:, 2:3], gt[:, 3:4], gt[:, 1:2])
                wb = wk.tile([128, 1], F32, name=f"wb{i}")
                nc.vector.tensor_sub(wb[:], gt[:, 2:3], gt[:, 0:1])
                nc.scalar.activation(cg[:, 3:4], cg[:, 2:3], AF.Ln)
                nc.scalar.activation(cg[:, 4:5], wb[:], AF.Ln)
                nc.vector.tensor_copy(cg[:, 5:6], gt[:, 4:5])
                nc.vector.tensor_copy(cg[:, 6:7], invc[i][:])
                nc.sync.dma_start(cstage[:, half].transpose((1, 0)), cg[:])
            bcall = wk.tile([AL, 7 * B], F32)
            nc.sync.dma_start(
                bcall[:], cstage[:, :].partition_broadcast(AL)
                .rearrange("p r b -> p (r b)"))
            bc = {nm: bcall[:, B * r:B * (r + 1)]
                  for r, nm in enumerate(["xb", "yb", "hb", "lnhb",
                                          "lnwb", "tb", "invc"])}

            # ---------------- track-side geometry scalars ----------------
            xt = wk.tile([AL, 1], F32)
            nc.vector.tensor_add(xt[:], trkg_t[:, 0:1], trkg_t[:, 2:3])
            nc.vector.tensor_scalar_mul(xt[:], xt[:], 0.5)
            yt = wk.tile([AL, 1], F32)
            nc.vector.tensor_add(yt[:], trkg_t[:, 1:2], trkg_t[:, 3:4])
            nc.vector.tensor_scalar_mul(yt[:], yt[:], 0.5)
            ht = wk.tile([AL, 1], F32)
            nc.vector.tensor_sub(ht[:], trkg_t[:, 3:4], trkg_t[:, 1:2])
            wt = wk.tile([AL, 1], F32)
            nc.vector.tensor_sub(wt[:], trkg_t[:, 2:3], trkg_t[:, 0:1])
            lnht = wk.tile([AL, 1], F32)
            nc.scalar.activation(lnht[:], ht[:], AF.Ln)
            lnwt = wk.tile([AL, 1], F32)
            nc.scalar.activation(lnwt[:], wt[:], AF.Ln)

            # ---------------- edge features (AL, B) each ----------------
            den = wk.tile([AL, B], F32)
            nc.vector.tensor_scalar_add(den[:], bc["hb"][:], ht[:, 0:1])
            rden = wk.tile([AL, B], F32)
            nc.vector.reciprocal(rden[:], den[:])

            feats = []
            f0 = wk.tile([AL, B], F32, name="f_x")
            nc.vector.tensor_scalar(f0[:], bc["xb"][:], xt[:, 0:1], 2.0,
                                    ALU.subtract, ALU.mult)
            nc.vector.tensor_mul(f0[:], f0[:], rden[:])
            feats.append(f0)
            f1 = wk.tile([AL, B], F32, name="f_y")
            nc.vector.tensor_scalar(f1[:], bc["yb"][:], yt[:, 0:1], 2.0,
                                    ALU.subtract, ALU.mult)
            nc.vector.tensor_mul(f1[:], f1[:], rden[:])
            feats.append(f1)
            f2 = wk.tile([AL, B], F32, name="f_w")
            nc.vector.tensor_scalar(f2[:], bc["lnwb"][:], -1.0, lnwt[:, 0:1],
                                    ALU.mult, ALU.add)
            feats.append(f2)
            f3 = wk.tile([AL, B], F32, name="f_h")
            nc.vector.tensor_scalar(f3[:], bc["lnhb"][:], -1.0, lnht[:, 0:1],
                                    ALU.mult, ALU.add)
            feats.append(f3)
            f4 = wk.tile([AL, B], F32, name="f_t")
            nc.vector.tensor_scalar_sub(f4[:], bc["tb"][:], trkg_t[:, 4:5])
            feats.append(f4)

            # dist_reid = 1 - gram * inv_t * inv_c
            pg = pp3.tile([AL, B], F32, tag="p3")
            for j in range(4):
                nc.tensor.matmul(pg[:], tf_t[:, AL * j:AL * (j + 1)],
                                 cf_t[:, B * j:B * (j + 1)],
                                 start=(j == 0), stop=(j == 3))
            f5 = wk.tile([AL, B], F32, name="f_d")
            nc.vector.tensor_scalar(f5[:], pg[:], inv_t[:, 0:1], None,
                                    ALU.mult)
            nc.vector.tensor_mul(f5[:], f5[:], bc["invc"][:])
            nc.scalar.activation(f5[:], f5[:], AF.Copy, bias=1.0, scale=-1.0)
            feats.append(f5)

            # ---------------- transpose features -> efT (6, 8192) ----------
            ef_stage = dr.tile([6, NP], F32R)
            for f, t in enumerate(feats):
                fr = wk.tile([AL, B], F32R, name=f"fr{f}", tag="fr",
                             bufs=2)
                nc.vector.tensor_copy(fr[:], t[:])
                nc.sync.dma_start(
                    ef_stage[f:f + 1, :].rearrange("o (a b) -> (o a) b", a=AL),
                    fr[:])
            upds = [st.tile([128, NP], F32R, name="updA"),
                    st.tile([128, NP], F32R, name="updB")]
            efT_t = upds[0][0:6, :]
            nc.sync.dma_start(efT_t, ef_stage[:])

            # ---------------- weight / bias loads ----------------
            we1_t = cp.tile([128, 4 * D], F32R)
            wlin_t = cp.tile([128, 4 * D], F32R)
            wn1_t = cp.tile([128, 2 * D], F32R)
            we1_s = wk.tile([128, 4 * D], F32, tag="wstage", bufs=2)
            wlin_s = wk.tile([128, 4 * D], F32, tag="wstage", bufs=2)
            wn1_s = wk.tile([128, 2 * D], F32, tag="wstage", bufs=2)
            for j in range(4):
                nc.scalar.dma_start(we1_s[:, 128 * j:128 * (j + 1)],
                                    we1T[128 * j:128 * (j + 1), :])
                nc.scalar.dma_start(wlin_s[:, 128 * j:128 * (j + 1)],
                                    wlinT[128 * j:128 * (j + 1), :])
            for j in range(2):
                nc.scalar.dma_start(wn1_s[:, 128 * j:128 * (j + 1)],
                                    wn1T[128 * j:128 * (j + 1), :])
            nc.vector.tensor_copy(we1_t[:], we1_s[:])
            nc.vector.tensor_copy(wlin_t[:], wlin_s[:])
            nc.vector.tensor_copy(wn1_t[:], wn1_s[:])
            wein1_t = cp.tile([6, D], F32R)
            wein2_t = cp.tile([128, D], F32R)
            we2_t = cp.tile([128, D], F32R)
            wn2_t = cp.tile([128, D], F32R)
            wc1_t = cp.tile([128, D], F32R)
            wc2c_t = cp.tile([128, 1], F32R)
            for dst, src in [(wein1_t, wein1T), (wein2_t, wein2T),
                             (we2_t, we2T), (wn2_t, wn2T), (wc1_t, wc1T),
                             (wc2c_t, wc2c)]:
                s = wk.tile(list(dst.shape), F32, name=f"ws_{src.name}",
                            tag="wstage", bufs=2)
                nc.scalar.dma_start(s[:], src[:])
                nc.vector.tensor_copy(dst[:], s[:])
            # combined fixed+upd K-tile for step 0 (upd == fixed there)
            wfu_t = cp.tile([128, D], F32R)
            nc.vector.tensor_add(wfu_t[:], we1_t[:, 2 * D:3 * D],
                                 we1_t[:, 3 * D:4 * D])
            biases = {}
            for nm, src in [("blin", blin), ("bein1", bein1), ("bein2", bein2),
                            ("be1", be1), ("be2", be2), ("bn1", bn1),
                            ("bn2", bn2), ("bc1", bc1)]:
                t = cp.tile([128, 1], F32, name=f"b_{nm}")
                nc.scalar.dma_start(t[:], src[:])
                biases[nm] = t

            # ---------------- fixed_edge = mlp2(edge_feats) ----------------
            fixedT = st.tile([128, NP], F32R)
            for c in range(NCH):
                sl = slice(CH * c, CH * (c + 1))
                p1 = pp1.tile([128, CH], F32, tag="p1")
                nc.tensor.matmul(p1[:], wein1_t[:], efT_t[:, sl],
                                 start=True, stop=True)
                h = wk.tile([128, CH], F32R, tag="h1", bufs=2)
                if c % 2 == 0:
                    nc.scalar.activation(h[:], p1[:], AF.Relu,
                                         bias=biases["bein1"][:, 0:1])
                else:
                    nc.vector.tensor_scalar(h[:], p1[:],
                                            biases["bein1"][:, 0:1], 0.0,
                                            ALU.add, ALU.max)
                p2 = pp2.tile([128, CH], F32, tag="p2")
                nc.tensor.matmul(p2[:], wein2_t[:], h[:], start=True, stop=True)
                if c % 2 == 0:
                    nc.vector.tensor_scalar(fixedT[:, sl], p2[:],
                                            biases["bein2"][:, 0:1], 0.0,
                                            ALU.add, ALU.max)
                else:
                    nc.scalar.activation(fixedT[:, sl], p2[:], AF.Relu,
                                         bias=biases["bein2"][:, 0:1])

            # ---------------- initial node embeds ----------------
            pt = pp3.tile([128, AL], F32, tag="p3")
            for j in range(4):
                nc.tensor.matmul(pt[:], wlin_t[:, 128 * j:128 * (j + 1)],
                                 tf_t[:, AL * j:AL * (j + 1)],
                                 start=(j == 0), stop=(j == 3))
            te = [st.tile([128, AL], F32R, name="teA"),
                  st.tile([128, AL], F32R, name="teB")]
            nc.scalar.activation(te[0][:], pt[:], AF.Relu,
                                 bias=biases["blin"][:, 0:1])
            pc = pp3.tile([128, B], F32, tag="p3")
            for j in range(4):
                nc.tensor.matmul(pc[:], wlin_t[:, 128 * j:128 * (j + 1)],
                                 cf_t[:, B * j:B * (j + 1)],
                                 start=(j == 0), stop=(j == 3))
            ce = [st.tile([128, B], F32R, name="ceA"),
                  st.tile([128, B], F32R, name="ceB")]
            nc.scalar.activation(ce[0][:], pc[:], AF.Relu,
                                 bias=biases["blin"][:, 0:1])

            # ---------------- step-loop state tiles ----------------
            msga = [st.tile([128, AL], F32R, name="msgaA"),
                    st.tile([128, AL], F32R, name="msgaB")]
            we1_te = we1_t[:, 0:D]
            we1_ce = we1_t[:, D:2 * D]
            we1_up = we1_t[:, 2 * D:3 * D]
            we1_fx = we1_t[:, 3 * D:4 * D]
            wn1_a = wn1_t[:, 0:D]
            wn1_b = wn1_t[:, D:2 * D]

            def cls_front(u_src, kout, c):
                """wc1 + hc for classifier chunk c of step kout."""
                sl = slice(CH * c, CH * (c + 1))
                p3 = pp3.tile([128, CH], F32, tag="p3")
                nc.tensor.matmul(p3[:], wc1_t[:], u_src[:, sl],
                                 start=True, stop=True)
                hc = wk.tile([128, CH], F32R, tag="hc", bufs=4,
                             name=f"hc{kout}_{c}")
                if c % 2 == 0:
                    nc.scalar.activation(hc[:], p3[:], AF.Relu,
                                         bias=biases["bc1"][:, 0:1])
                else:
                    nc.vector.tensor_scalar(hc[:], p3[:],
                                            biases["bc1"][:, 0:1], 0.0,
                                            ALU.add, ALU.max)
                return hc

            def cls_back(kout, c, hc):
                """wc2 + logits staging + DMA for classifier chunk c."""
                plg = plgp.tile([1, CH], F32, tag="plg",
                                name=f"plg{kout}_{c}")
                nc.tensor.matmul(plg[:], wc2c_t[:], hc[:],
                                 start=True, stop=True)
                lg = wk.tile([1, CH], F32, tag="lg", bufs=6,
                             name=f"lg{kout}_{c}")
                if c % 2 == 0:
                    nc.vector.tensor_copy(lg[:], plg[:])
                else:
                    nc.scalar.activation(lg[:], plg[:], AF.Copy)
                nc.sync.dma_start(out[kout:kout + 1,
                                      CH * c:CH * (c + 1)], lg[:])

            arbufs = {}
            for k in range(STEPS):
                u_prev = fixedT if k == 0 else upds[(k + 1) % 2]
                u_cur = upds[k % 2]
                te_prev, te_cur = te[(k + 1) % 2], te[k % 2]
                ce_prev, ce_cur = ce[(k + 1) % 2], ce[k % 2]
                last = k == STEPS - 1

                def s1_pair(c0):
                    """S1 for chunks c0, c0+1 with same-weight matmuls
                    adjacent so LDWEIGHTS pre-loads and the second matmul
                    streams at full rate."""
                    sl0 = slice(CH * c0, CH * (c0 + 1))
                    sl1 = slice(CH * (c0 + 1), CH * (c0 + 2))
                    pa = pp1.tile([128, CH], F32, tag="p1",
                                  name=f"p1_{k}_{c0}")
                    pb = pp1.tile([128, CH], F32, tag="p1",
                                  name=f"p1_{k}_{c0 + 1}")
                    if k == 0:
                        nc.tensor.matmul(pa[:], wfu_t[:], fixedT[:, sl0],
                                         start=True, stop=True)
                        nc.tensor.matmul(pb[:], wfu_t[:], fixedT[:, sl1],
                                         start=True, stop=True)
                    else:
                        nc.tensor.matmul(pa[:], we1_fx, fixedT[:, sl0],
                                         start=True, stop=False)
                        nc.tensor.matmul(pb[:], we1_fx, fixedT[:, sl1],
                                         start=True, stop=False)
                        nc.tensor.matmul(pa[:], we1_up, u_prev[:, sl0],
                                         start=False, stop=True)
                        nc.tensor.matmul(pb[:], we1_up, u_prev[:, sl1],
                                         start=False, stop=True)
                    p1ring[c0] = pa
                    p1ring[c0 + 1] = pb

                p1ring = {}
                s1_pair(0)

                cls_state = {"nf": 0, "pend": []}

                def emit_cls(n):
                    if k == 0:
                        return
                    fronts = []
                    while n > 0 and cls_state["nf"] < NCH:
                        c = cls_state["nf"]
                        fronts.append((c, cls_front(u_prev, k - 1, c)))
                        cls_state["nf"] = c + 1
                        n -= 1
                    cls_state["pend"].extend(fronts)
                    while len(cls_state["pend"]) > 1:
                        c, hcp = cls_state["pend"].pop(0)
                        cls_back(k - 1, c, hcp)

                def finish_cls():
                    while cls_state["pend"]:
                        c, hcp = cls_state["pend"].pop(0)
                        cls_back(k - 1, c, hcp)

                # classifier cover: all 16 chunks for k==1 (the first AR
                # is cold and needs a much bigger shadow), else 6
                NCLS0 = NCH if k == 1 else 8
                emit_cls(NCLS0)

                if k > 0:
                    # te(k) = mlp2([te(k-1), msga(k-1)])
                    pt1 = pp3.tile([128, AL], F32, tag="p3")
                    nc.tensor.matmul(pt1[:], wn1_a, te_prev[:],
                                     start=True, stop=False)
                    nc.tensor.matmul(pt1[:], wn1_b, msga[(k + 1) % 2][:],
                                     start=False, stop=True)
                    tn1 = wk.tile([128, AL], F32R, tag="tn1", bufs=2)
                    nc.scalar.activation(tn1[:], pt1[:], AF.Relu,
                                         bias=biases["bn1"][:, 0:1])
                    pt2 = pp3.tile([128, AL], F32, tag="p3")
                    nc.tensor.matmul(pt2[:], wn2_t[:], tn1[:],
                                     start=True, stop=True)
                    nc.scalar.activation(te_cur[:], pt2[:], AF.Relu,
                                         bias=biases["bn2"][:, 0:1])

                # Tb = W1a @ te(k) + be1  (h1 bias, (128, AL))
                pT = pp3.tile([128, AL], F32, tag="p3")
                nc.tensor.matmul(pT[:], we1_te, te_cur[:],
                                 start=True, stop=True)
                Tb = wk.tile([128, AL], F32, tag="tb", bufs=2)
                nc.scalar.activation(Tb[:], pT[:], AF.Identity,
                                     bias=biases["be1"][:, 0:1])

                # ===== AR(k-1)-dependent section =====
                # The ce-update chain alternates PE and scalar; classifier
                # chunks 6-8 are interleaved so the in-order PE queue has
                # independent work during the scalar hops.
                Cs = wk.tile([128, B], F32R, name=f"Cs{k}", tag="csb",
                             bufs=2)
                if k > 0:
                    mb_out_p, = arbufs.pop("out")
                    msgb_in = wk.tile([128, 2 * B], F32, tag="mbf", bufs=2)
                    nc.sync.dma_start(msgb_in[:, 0:B], mb_out_p[0:128, :])
                    nc.scalar.dma_start(msgb_in[:, B:2 * B],
                                        mb_out_p[128:256, :])
                    pc1 = pp3.tile([128, B], F32, tag="p3")
                    nc.tensor.matmul(pc1[:], wn1_a, ce_prev[:],
                                     start=True, stop=False)
                    nc.tensor.matmul(pc1[:], wn1_b,
                                     msgb_in[:, 0:B].bitcast(F32R),
                                     start=False, stop=False)
                    nc.tensor.matmul(pc1[:], wn1_b,
                                     msgb_in[:, B:2 * B].bitcast(F32R),
                                     start=False, stop=True)
                    cn1 = wk.tile([128, B], F32R, tag="cn1", bufs=2)
                    nc.scalar.activation(cn1[:], pc1[:], AF.Relu,
                                         bias=biases["bn1"][:, 0:1])
                    emit_cls(1)
                    pc2 = pp3.tile([128, B], F32, tag="p3")
                    nc.tensor.matmul(pc2[:], wn2_t[:], cn1[:],
                                     start=True, stop=True)
                    nc.scalar.activation(ce_cur[:], pc2[:], AF.Relu,
                                         bias=biases["bn2"][:, 0:1])
                    emit_cls(1)
                    pC = pp3.tile([128, B], F32, tag="p3")
                    nc.tensor.matmul(pC[:], we1_ce, ce_cur[:],
                                     start=True, stop=True)
                    nc.vector.tensor_copy(Cs[:], pC[:])
                    emit_cls(1)
                else:
                    pC = pp3.tile([128, B], F32, tag="p3")
                    nc.tensor.matmul(pC[:], we1_ce, ce_cur[:],
                                     start=True, stop=True)
                    nc.vector.tensor_copy(Cs[:], pC[:])

                accs = {}

                def do_pair(c):
                    g0 = u_cur[:, CH * c:CH * c + B].bitcast(F32)
                    g1 = u_cur[:, CH * c + B:CH * (c + 1)].bitcast(F32)
                    if c < 2:
                        acc = wk.tile([128, B], F32, tag="acc", bufs=4,
                                      name=f"acc{k}_{c}")
                        nc.vector.tensor_add(acc[:], g0, g1)
                        accs[c] = acc
                    else:
                        pr = wk.tile([128, B], F32, tag="pair", bufs=6,
                                     name=f"pr{k}_{c}")
                        nc.vector.tensor_add(pr[:], g0, g1)
                        nc.vector.tensor_add(accs[c % 2][:], accs[c % 2][:],
                                             pr[:])

                def msga_part(q):
                    # per-a sums over b for chunks 4q..4q+3 (contiguous)
                    seg = u_cur[:, 2048 * q:2048 * (q + 1)].bitcast(F32)
                    nc.vector.tensor_reduce(
                        msga_f[:, 8 * q:8 * (q + 1)],
                        seg.rearrange("p (a b) -> p a b", a=8),
                        mybir.AxisListType.X, ALU.add)

                if not last:
                    msga_f = wk.tile([128, AL], F32, tag="msgaf", bufs=2,
                                     name=f"msgaf{k}")

                for cp in range(0, NCH, 2):
                    ts = []
                    for c in (cp, cp + 1):
                        p1 = p1ring.pop(c)
                        t = wk.tile([128, CH], F32R, tag="t", bufs=4,
                                    name=f"t{k}_{c}")
                        for g in range(2):
                            bsl = slice(B * g, B * (g + 1))
                            col = 2 * c + g
                            nc.vector.scalar_tensor_tensor(
                                t[:, bsl], p1[:, bsl], Tb[:, col:col + 1],
                                Cs[:], ALU.add, ALU.add)
                        nc.scalar.activation(t[:], t[:], AF.Relu)
                        ts.append(t)
                    p2s = []
                    for t in ts:
                        p2 = pp2.tile([128, CH], F32, tag="p2")
                        nc.tensor.matmul(p2[:], we2_t[:], t[:],
                                         start=True, stop=True)
                        p2s.append(p2)
                    for i, c in enumerate((cp, cp + 1)):
                        sl = slice(CH * c, CH * (c + 1))
                        nc.scalar.activation(u_cur[:, sl], p2s[i][:],
                                             AF.Relu,
                                             bias=biases["be2"][:, 0:1])
                    if cp + 2 < NCH:
                        s1_pair(cp + 2)
                    emit_cls(2)
                    if not last and cp >= 2:
                        do_pair(cp - 2)
                        do_pair(cp - 1)
                        if cp in (6, 10, 14):
                            msga_part(cp // 4 - 1)
                finish_cls()

                if not last:
                    do_pair(NCH - 2)
                    do_pair(NCH - 1)
                    # the two accumulator halves are summed by the CCE in
                    # the AllReduce itself (payload is latency-flat) and
                    # by the ce-update's accumulating matmuls afterwards
                    mb_in = dr.tile([2 * 128, B], F32, tag="mbin", bufs=2)
                    mb_out = dr.tile([2 * 128, B], F32, tag="mbout", bufs=2,
                                     addr_space="Shared")
                    nc.sync.dma_start(mb_in[0:128, :], accs[0][:])
                    nc.scalar.dma_start(mb_in[128:256, :], accs[1][:])
                    nc.gpsimd.collective_compute(
                        "AllReduce", mybir.AluOpType.add,
                        replica_groups=[list(range(8))],
                        ins=[mb_in.opt()], outs=[mb_out.opt()])
                    arbufs["out"] = (mb_out,)
                    msga_part(3)
                    nc.vector.tensor_copy(msga[k % 2][:], msga_f[:])

            # final classifier for step 7
            u7 = upds[(STEPS - 1) % 2]
            hc_pend = None
            for c in range(NCH):
                hc_new = cls_front(u7, STEPS - 1, c)
                if hc_pend is not None:
                    cls_back(STEPS - 1, c - 1, hc_pend)
                hc_pend = hc_new
            cls_back(STEPS - 1, NCH - 1, hc_pend)

    nc.finalize()
    _BUILD_CACHE["nc"] = nc
    return nc


def _make_in_maps(inputs):
    f32 = np.float32

    def c(x):
        return np.ascontiguousarray(np.asarray(x, dtype=f32))

    tf = c(inputs["track_features"])
    cf = c(inputs["current_features"])
    tb = c(inputs["track_boxes"])
    cb = c(inputs["current_boxes"])
    tt = c(inputs["track_time"]).reshape(-1, 1)
    ct = c(inputs["current_time"]).reshape(-1, 1)

    shared = {
        "cfT": c(cf.T),
        "curf": cf,
        "curg": c(np.concatenate([cb, ct], axis=1)),
        "wlinT": c(inputs["w_lin"].T),
        "blin": c(np.broadcast_to(inputs["b_lin"][:, None], (D, 1))),
        "wein1T": c(inputs["w_ein1"].T),
        "bein1": c(inputs["b_ein1"][:, None]),
        "wein2T": c(inputs["w_ein2"].T),
        "bein2": c(inputs["b_ein2"][:, None]),
        "we1T": c(inputs["w_e1"].T),
        "be1": c(inputs["b_e1"][:, None]),
        "we2T": c(inputs["w_e2"].T),
        "be2": c(inputs["b_e2"][:, None]),
        "wn1T": c(inputs["w_n1"].T),
        "bn1": c(inputs["b_n1"][:, None]),
        "wn2T": c(inputs["w_n2"].T),
        "bn2": c(inputs["b_n2"][:, None]),
        "wc1T": c(inputs["w_c1"].T),
        "bc1": c(inputs["b_c1"][:, None]),
        "wc2c": c(inputs["w_c2"].T),
    }
    in_maps = []
    for core in range(8):
        rows = slice(AL * core, AL * (core + 1))
        m = dict(shared)
        m["tfT"] = c(tf[rows].T)
        m["trkf"] = c(tf[rows])
        m["trkg"] = c(np.concatenate([tb[rows], tt[rows]], axis=1))
        in_maps.append(m)
    return in_maps


def run(trace=False, trace_cores=None, **inputs):
    from concourse.bass_utils import run_bass_kernel_spmd

    if trace:
        _install_ntff_hook()
    nc = _build()
    in_maps = _make_in_maps(inputs)
    res = run_bass_kernel_spmd(nc, in_maps, core_ids=list(range(8)),
                               trace=trace, trace_cores=trace_cores)
    full = np.empty((STEPS, A, B), np.float32)
    for core in range(8):
        full[:, AL * core:AL * (core + 1), :] = \
            res.results[core]["out"].reshape(STEPS, AL, B)
    full += np.asarray(inputs["b_c2"], np.float32).reshape(1, 1, 1)
    return full, res


def kernel(**inputs):
    full, _ = run(trace=False, **inputs)
    return full


def _install_ntff_hook():
    import sys
    import types
    try:
        from antenv.axon_hooks import get_axon_ntff_profile_hook  # noqa: F401
        return
    except ImportError:
        pass
    import antenv
    from trn_agent_boot.trn_boot import _ntff_profile_via_ctypes

    mod = types.ModuleType("antenv.axon_hooks")
    holder = [_ntff_profile_via_ctypes("/opt/axon/libaxon_pjrt.so")]
    mod.get_axon_ntff_profile_hook = lambda: holder[0]
    mod.set_axon_ntff_profile_hook = lambda h: holder.__setitem__(0, h)
    sys.modules["antenv.axon_hooks"] = mod
    antenv.axon_hooks = mod


# revision 29
# speedup vs baseline: 1.0812x; 1.0812x over previous
"""AssignmentSimilarityNet GNN message-passing kernel for 8 Trainium2
NeuronCores.

Sharding: track (A) dimension split across 8 cores (32 tracks each).
Edge tensors, track embeds and messages-to-A stay local; messages-to-B
(sum over A) are all-reduced each step; MLP weights replicated.

Schedule (per step, steady state) — built around the ~23us serial
latency of one 8-core AllReduce (collectives cannot be pipelined in
the CC engine, so exactly one AR per step, fully covered):

  cover section (AR(k-1) in flight):
    - te(k) node update (local, from msga(k-1)); Tb = W1a@te + be1
    - S1 prologue: first two chunks' fixed+upd K-tile matmuls into PSUM
    - classifier chunks 0..5 of step k-1, software-pipelined on the PE
      (wc1 of chunk c+1 issues before the M=1 wc2 of chunk c); logits
      land in (1,512) PSUM rows, staged to SBUF (vector/scalar
      alternating), one small DMA per chunk
  AR-dependent section:
    - fetch msgb (split across sync+scalar queues), ce(k) node update
      with classifier chunks 6..8 interleaved into the PE queue so the
      in-order queue has work during the scalar hops; C = W1b @ ce(k)
    - per chunk: t = (p1 + Tb_col) + C via two scalar_tensor_tensor ops
      (vector), relu in place (scalar), we2 matmul, u = relu(p2+be2)
      (scalar); remaining S1 and classifier chunks interleaved
    - msgb via contiguous pair adds + two accumulator chains (vector);
      msga via 4 contiguous strided-view reduces off the critical path;
      stage DMA, trigger AR(k) (gpsimd queue holds only the triggers)
  The gpsimd/Pool engine is useless for bulk elementwise work (~9us for
  a 512-wide relu) and cannot touch PSUM; it only issues the collective
  triggers so no compute queue ever blocks on an AR.

The 8th AR is skipped (last node update is dead).  b_c2 is added on the
host.  All compute is fp32/fp32r: bf16 anywhere in the recurrent or
message path overshoots the 2e-2 error budget (measured 2.6e-2 for a
bf16 msgb alone; the net amplifies per-op rounding ~100x over 8 steps).
"""
import numpy as np

A = 256          # tracks
B = 256          # current detections
AL = A // 8      # tracks per core (32)
REID = 512
D = 128          # ND == ED
STEPS = 8
NP = AL * B      # pairs per core (8192)
CH = 512         # pair chunk (2 a-groups x 256 b)
NCH = NP // CH   # 16

_BUILD_CACHE = {}


def _build():
    if "nc" in _BUILD_CACHE:
        return _BUILD_CACHE["nc"]
    import concourse.bacc as bacc
    import concourse.mybir as mybir
    import concourse.tile as tile

    F32 = mybir.dt.float32
    F32R = mybir.dt.float32r
    AF = mybir.ActivationFunctionType
    ALU = mybir.AluOpType

    nc = bacc.Bacc(None, target_bir_lowering=False)

    def din(name, shape):
        return nc.dram_tensor(name, shape, F32, kind="ExternalInput")

    tfT = din("tfT", [REID, AL])
    trkf = din("trkf", [AL, REID])
    cfT = din("cfT", [REID, B])
    curf = din("curf", [B, REID])
    trkg = din("trkg", [AL, 5])
    curg = din("curg", [B, 5])
    wlinT = din("wlinT", [REID, D])
    blin = din("blin", [D, 1])
    wein1T = din("wein1T", [6, D])
    bein1 = din("bein1", [D, 1])
    wein2T = din("wein2T", [D, D])
    bein2 = din("bein2", [D, 1])
    we1T = din("we1T", [4 * D, D])
    be1 = din("be1", [D, 1])
    we2T = din("we2T", [D, D])
    be2 = din("be2", [D, 1])
    wn1T = din("wn1T", [2 * D, D])
    bn1 = din("bn1", [D, 1])
    wn2T = din("wn2T", [D, D])
    bn2 = din("bn2", [D, 1])
    wc1T = din("wc1T", [D, D])
    bc1 = din("bc1", [D, 1])
    wc2c = din("wc2c", [D, 1])
    out = nc.dram_tensor("out", [STEPS, NP], F32, kind="ExternalOutput")

    with tile.TileContext(nc) as tc:
        with (
            tc.tile_pool(name="const", bufs=1) as cp,
            tc.tile_pool(name="state", bufs=1) as st,
            tc.tile_pool(name="work", bufs=1) as wk,
            tc.tile_pool(name="p1", bufs=2, space="PSUM") as pp1,
            tc.tile_pool(name="p2", bufs=2, space="PSUM") as pp2,
            tc.tile_pool(name="p3", bufs=2, space="PSUM") as pp3,
            tc.tile_pool(name="plg", bufs=2, space="PSUM") as plgp,
            tc.tile_pool(name="dram", bufs=1, space="DRAM") as dr,
        ):
            # ---------------- feature loads ----------------
            tf_t = st.tile([128, 4 * AL], F32R)       # 4 K-tiles of (128, 32)
            cf_t = st.tile([128, 4 * B], F32R)        # 4 K-tiles of (128, 256)
            tf_s = wk.tile([128, 4 * AL], F32, tag="wstage", bufs=2)
            cf_s = wk.tile([128, 4 * B], F32, tag="wstage", bufs=2)
            for j in range(4):
                nc.sync.dma_start(tf_s[:, AL * j:AL * (j + 1)],
                                  tfT[128 * j:128 * (j + 1), :])
                nc.sync.dma_start(cf_s[:, B * j:B * (j + 1)],
                                  cfT[128 * j:128 * (j + 1), :])
            for j in range(4):
                nc.vector.tensor_copy(tf_t[:, AL * j:AL * (j + 1)],
                                      tf_s[:, AL * j:AL * (j + 1)])
                nc.vector.tensor_copy(cf_t[:, B * j:B * (j + 1)],
                                      cf_s[:, B * j:B * (j + 1)])
            trkf_t = wk.tile([AL, REID], F32)
            nc.scalar.dma_start(trkf_t[:], trkf[:])
            curf_t0 = wk.tile([128, REID], F32)
            curf_t1 = wk.tile([128, REID], F32)
            nc.scalar.dma_start(curf_t0[:], curf[0:128, :])
            nc.scalar.dma_start(curf_t1[:], curf[128:256, :])
            trkg_t = wk.tile([AL, 5], F32)
            nc.scalar.dma_start(trkg_t[:], trkg[:])
            curg_t0 = wk.tile([128, 5], F32)
            curg_t1 = wk.tile([128, 5], F32)
            nc.scalar.dma_start(curg_t0[:], curg[0:128, :])
            nc.scalar.dma_start(curg_t1[:], curg[128:256, :])

            # ---------------- reid norms ----------------
            sq_t = wk.tile([AL, REID], F32, tag="sq", bufs=1)
            nc.vector.tensor_mul(sq_t[:], trkf_t[:], trkf_t[:])
            sst = wk.tile([AL, 1], F32)
            nc.vector.tensor_reduce(sst[:], sq_t[:], mybir.AxisListType.X, ALU.add)
            rt = wk.tile([AL, 1], F32)
            nc.vector.reciprocal(rt[:], sst[:])
            inv_t = wk.tile([AL, 1], F32)
            nc.scalar.activation(inv_t[:], rt[:], AF.Sqrt)

            invc = []
            for i, ct in enumerate((curf_t0, curf_t1)):
                sq_c = wk.tile([128, REID], F32, name=f"sq_c{i}", tag="sq", bufs=1)
                nc.vector.tensor_mul(sq_c[:], ct[:], ct[:])
                ssc = wk.tile([128, 1], F32, name=f"ssc{i}")
                nc.vector.tensor_reduce(ssc[:], sq_c[:], mybir.AxisListType.X,
                                        ALU.add)
                rc = wk.tile([128, 1], F32, name=f"rc{i}")
                nc.vector.reciprocal(rc[:], ssc[:])
                ic = wk.tile([128, 1], F32, name=f"ic{i}")
                nc.scalar.activation(ic[:], rc[:], AF.Sqrt)
                invc.append(ic)

            # ---------------- current-side geometry -> bcast rows ----------
            # rows of cstage: 0 xb, 1 yb, 2 hb, 3 ln hb, 4 ln wb, 5 tb, 6 invc
            cstage = dr.tile([7, B], F32)
            for i, gt in enumerate((curg_t0, curg_t1)):
                half = slice(128 * i, 128 * (i + 1))
                cg = wk.tile([128, 7], F32, name=f"cg{i}")
                nc.vector.tensor_add(cg[:, 0:1], gt[:, 0:1], gt[:, 2:3])
                nc.vector.tensor_scalar_mul(cg[:, 0:1], cg[:, 0:1], 0.5)
                nc.vector.tensor_add(cg[:, 1:2], gt[:, 1:2], gt[:, 3:4])
                nc.vector.tensor_scalar_mul(cg[:, 1:2], cg[:, 1:2], 0.5)
                nc.vector.tensor_sub(cg[:, 2:3], gt[:, 3:4], gt[:, 1:2])
                wb = wk.tile([128, 1], F32, name=f"wb{i}")
                nc.vector.tensor_sub(wb[:], gt[:, 2:3], gt[:, 0:1])
                nc.scalar.activation(cg[:, 3:4], cg[:, 2:3], AF.Ln)
                nc.scalar.activation(cg[:, 4:5], wb[:], AF.Ln)
                nc.vector.tensor_copy(cg[:, 5:6], gt[:, 4:5])
                nc.vector.tensor_copy(cg[:, 6:7], invc[i][:])
                nc.sync.dma_start(cstage[:, half].transpose((1, 0)), cg[:])
            bcall = wk.tile([AL, 7 * B], F32)
            nc.sync.dma_start(
                bcall[:], cstage[:, :].partition_broadcast(AL)
                .rearrange("p r b -> p (r b)"))
            bc = {nm: bcall[:, B * r:B * (r + 1)]
                  for r, nm in enumerate(["xb", "yb", "hb", "lnhb",
                                          "lnwb", "tb", "invc"])}

            # ---------------- track-side geometry scalars ----------------
            xt = wk.tile([AL, 1], F32)
            nc.vector.tensor_add(xt[:], trkg_t[:, 0:1], trkg_t[:, 2:3])
            nc.vector.tensor_scalar_mul(xt[:], xt[:], 0.5)
            yt = wk.tile([AL, 1], F32)
            nc.vector.tensor_add(yt[:], trkg_t[:, 1:2], trkg_t[:, 3:4])
            nc.vector.tensor_scalar_mul(yt[:], yt[:], 0.5)
            ht = wk.tile([AL, 1], F32)
            nc.vector.tensor_sub(ht[:], trkg_t[:, 3:4], trkg_t[:, 1:2])
            wt = wk.tile([AL, 1], F32)
            nc.vector.tensor_sub(wt[:], trkg_t[:, 2:3], trkg_t[:, 0:1])
            lnht = wk.tile([AL, 1], F32)
            nc.scalar.activation(lnht[:], ht[:], AF.Ln)
            lnwt = wk.tile([AL, 1], F32)
            nc.scalar.activation(lnwt[:], wt[:], AF.Ln)

            # ---------------- edge features (AL, B) each ----------------
            den = wk.tile([AL, B], F32)
            nc.vector.tensor_scalar_add(den[:], bc["hb"][:], ht[:, 0:1])
            rden = wk.tile([AL, B], F32)
            nc.vector.reciprocal(rden[:], den[:])

            feats = []
            f0 = wk.tile([AL, B], F32, name="f_x")
            nc.vector.tensor_scalar(f0[:], bc["xb"][:], xt[:, 0:1], 2.0,
                                    ALU.subtract, ALU.mult)
            nc.vector.tensor_mul(f0[:], f0[:], rden[:])
            feats.append(f0)
            f1 = wk.tile([AL, B], F32, name="f_y")
            nc.vector.tensor_scalar(f1[:], bc["yb"][:], yt[:, 0:1], 2.0,
                                    ALU.subtract, ALU.mult)
            nc.vector.tensor_mul(f1[:], f1[:], rden[:])
            feats.append(f1)
            f2 = wk.tile([AL, B], F32, name="f_w")
            nc.vector.tensor_scalar(f2[:], bc["lnwb"][:], -1.0, lnwt[:, 0:1],
                                    ALU.mult, ALU.add)
            feats.append(f2)
            f3 = wk.tile([AL, B], F32, name="f_h")
            nc.vector.tensor_scalar(f3[:], bc["lnhb"][:], -1.0, lnht[:, 0:1],
                                    ALU.mult, ALU.add)
            feats.append(f3)
            f4 = wk.tile([AL, B], F32, name="f_t")
            nc.vector.tensor_scalar_sub(f4[:], bc["tb"][:], trkg_t[:, 4:5])
            feats.append(f4)

            # dist_reid = 1 - gram * inv_t * inv_c
            pg = pp3.tile([AL, B], F32, tag="p3")
            for j in range(4):
                nc.tensor.matmul(pg[:], tf_t[:, AL * j:AL * (j + 1)],
                                 cf_t[:, B * j:B * (j + 1)],
                                 start=(j == 0), stop=(j == 3))
            f5 = wk.tile([AL, B], F32, name="f_d")
            nc.vector.tensor_scalar(f5[:], pg[:], inv_t[:, 0:1], None,
                                    ALU.mult)
            nc.vector.tensor_mul(f5[:], f5[:], bc["invc"][:])
            nc.scalar.activation(f5[:], f5[:], AF.Copy, bias=1.0, scale=-1.0)
            feats.append(f5)

            # ---------------- transpose features -> efT (6, 8192) ----------
            ef_stage = dr.tile([6, NP], F32R)
            for f, t in enumerate(feats):
                fr = wk.tile([AL, B], F32R, name=f"fr{f}", tag="fr",
                             bufs=2)
                nc.vector.tensor_copy(fr[:], t[:])
                nc.sync.dma_start(
                    ef_stage[f:f + 1, :].rearrange("o (a b) -> (o a) b", a=AL),
                    fr[:])
            upds = [st.tile([128, NP], F32R, name="updA"),
                    st.tile([128, NP], F32R, name="updB")]
            efT_t = upds[0][0:6, :]
            nc.sync.dma_start(efT_t, ef_stage[:])

            # ---------------- weight / bias loads ----------------
            we1_t = cp.tile([128, 4 * D], F32R)
            wlin_t = cp.tile([128, 4 * D], F32R)
            wn1_t = cp.tile([128, 2 * D], F32R)
            we1_s = wk.tile([128, 4 * D], F32, tag="wstage", bufs=2)
            wlin_s = wk.tile([128, 4 * D], F32, tag="wstage", bufs=2)
            wn1_s = wk.tile([128, 2 * D], F32, tag="wstage", bufs=2)
            for j in range(4):
                nc.scalar.dma_start(we1_s[:, 128 * j:128 * (j + 1)],
                                    we1T[128 * j:128 * (j + 1), :])
                nc.scalar.dma_start(wlin_s[:, 128 * j:128 * (j + 1)],
                                    wlinT[128 * j:128 * (j + 1), :])
            for j in range(2):
                nc.scalar.dma_start(wn1_s[:, 128 * j:128 * (j + 1)],
                                    wn1T[128 * j:128 * (j + 1), :])
            nc.vector.tensor_copy(we1_t[:], we1_s[:])
            nc.vector.tensor_copy(wlin_t[:], wlin_s[:])
            nc.vector.tensor_copy(wn1_t[:], wn1_s[:])
            wein1_t = cp.tile([6, D], F32R)
            wein2_t = cp.tile([128, D], F32R)
            we2_t = cp.tile([128, D], F32R)
            wn2_t = cp.tile([128, D], F32R)
            wc1_t = cp.tile([128, D], F32R)
            wc2c_t = cp.tile([128, 1], F32R)
            for dst, src in [(wein1_t, wein1T), (wein2_t, wein2T),
                             (we2_t, we2T), (wn2_t, wn2T), (wc1_t, wc1T),
                             (wc2c_t, wc2c)]:
                s = wk.tile(list(dst.shape), F32, name=f"ws_{src.name}",
                            tag="wstage", bufs=2)
                nc.scalar.dma_start(s[:], src[:])
                nc.vector.tensor_copy(dst[:], s[:])
            # combined fixed+upd K-tile for step 0 (upd == fixed there)
            wfu_t = cp.tile([128, D], F32R)
            nc.vector.tensor_add(wfu_t[:], we1_t[:, 2 * D:3 * D],
                                 we1_t[:, 3 * D:4 * D])
            biases = {}
            for nm, src in [("blin", blin), ("bein1", bein1), ("bein2", bein2),
                            ("be1", be1), ("be2", be2), ("bn1", bn1),
                            ("bn2", bn2), ("bc1", bc1)]:
                t = cp.tile([128, 1], F32, name=f"b_{nm}")
                nc.scalar.dma_start(t[:], src[:])
                biases[nm] = t

            # ---------------- fixed_edge = mlp2(edge_feats) ----------------
            fixedT = st.tile([128, NP], F32R)
            for c in range(NCH):
                sl = slice(CH * c, CH * (c + 1))
                p1 = pp1.tile([128, CH], F32, tag="p1")
                nc.tensor.matmul(p1[:], wein1_t[:], efT_t[:, sl],
                                 start=True, stop=True)
                h = wk.tile([128, CH], F32R, tag="h1", bufs=2)
                if c % 2 == 0:
                    nc.scalar.activation(h[:], p1[:], AF.Relu,
                                         bias=biases["bein1"][:, 0:1])
                else:
                    nc.vector.tensor_scalar(h[:], p1[:],
                                            biases["bein1"][:, 0:1], 0.0,
                                            ALU.add, ALU.max)
                p2 = pp2.tile([128, CH], F32, tag="p2")
                nc.tensor.matmul(p2[:], wein2_t[:], h[:], start=True, stop=True)
                if c % 2 == 0:
                    nc.vector.tensor_scalar(fixedT[:, sl], p2[:],
                                            biases["bein2"][:, 0:1], 0.0,
                                            ALU.add, ALU.max)
                else:
                    nc.scalar.activation(fixedT[:, sl], p2[:], AF.Relu,
                                         bias=biases["bein2"][:, 0:1])

            # ---------------- initial node embeds ----------------
            pt = pp3.tile([128, AL], F32, tag="p3")
            for j in range(4):
                nc.tensor.matmul(pt[:], wlin_t[:, 128 * j:128 * (j + 1)],
                                 tf_t[:, AL * j:AL * (j + 1)],
                                 start=(j == 0), stop=(j == 3))
            te = [st.tile([128, AL], F32R, name="teA"),
                  st.tile([128, AL], F32R, name="teB")]
            nc.scalar.activation(te[0][:], pt[:], AF.Relu,
                                 bias=biases["blin"][:, 0:1])
            pc = pp3.tile([128, B], F32, tag="p3")
            for j in range(4):
                nc.tensor.matmul(pc[:], wlin_t[:, 128 * j:128 * (j + 1)],
                                 cf_t[:, B * j:B * (j + 1)],
                                 start=(j == 0), stop=(j == 3))
            ce = [st.tile([128, B], F32R, name="ceA"),
                  st.tile([128, B], F32R, name="ceB")]
            nc.scalar.activation(ce[0][:], pc[:], AF.Relu,
                                 bias=biases["blin"][:, 0:1])

            # ---------------- step-loop state tiles ----------------
            msga = [st.tile([128, AL], F32R, name="msgaA"),
                    st.tile([128, AL], F32R, name="msgaB")]
            we1_te = we1_t[:, 0:D]
            we1_ce = we1_t[:, D:2 * D]
            we1_up = we1_t[:, 2 * D:3 * D]
            we1_fx = we1_t[:, 3 * D:4 * D]
            wn1_a = wn1_t[:, 0:D]
            wn1_b = wn1_t[:, D:2 * D]

            def cls_front(u_src, kout, c):
                """wc1 + hc for classifier chunk c of step kout."""
                sl = slice(CH * c, CH * (c + 1))
                p3 = pp3.tile([128, CH], F32, tag="p3")
                nc.tensor.matmul(p3[:], wc1_t[:], u_src[:, sl],
                                 start=True, stop=True)
                hc = wk.tile([128, CH], F32R, tag="hc", bufs=4,
                             name=f"hc{kout}_{c}")
                if c % 2 == 0:
                    nc.scalar.activation(hc[:], p3[:], AF.Relu,
                                         bias=biases["bc1"][:, 0:1])
                else:
                    nc.vector.tensor_scalar(hc[:], p3[:],
                                            biases["bc1"][:, 0:1], 0.0,
                                            ALU.add, ALU.max)
                return hc

            def cls_back(kout, c, hc):
                """wc2 + logits staging + DMA for classifier chunk c."""
                plg = plgp.tile([1, CH], F32, tag="plg",
                                name=f"plg{kout}_{c}")
                nc.tensor.matmul(plg[:], wc2c_t[:], hc[:],
                                 start=True, stop=True)
                lg = wk.tile([1, CH], F32, tag="lg", bufs=6,
                             name=f"lg{kout}_{c}")
                if c % 4 == 0:
                    nc.vector.tensor_copy(lg[:], plg[:])
                else:
                    nc.scalar.activation(lg[:], plg[:], AF.Copy)
                nc.sync.dma_start(out[kout:kout + 1,
                                      CH * c:CH * (c + 1)], lg[:])

            arbufs = {}
            for k in range(STEPS):
                u_prev = fixedT if k == 0 else upds[(k + 1) % 2]
                u_cur = upds[k % 2]
                te_prev, te_cur = te[(k + 1) % 2], te[k % 2]
                ce_prev, ce_cur = ce[(k + 1) % 2], ce[k % 2]
                last = k == STEPS - 1

                def s1_pair(c0):
                    """S1 for chunks c0, c0+1 with same-weight matmuls
                    adjacent so LDWEIGHTS pre-loads and the second matmul
                    streams at full rate."""
                    sl0 = slice(CH * c0, CH * (c0 + 1))
                    sl1 = slice(CH * (c0 + 1), CH * (c0 + 2))
                    pa = pp1.tile([128, CH], F32, tag="p1",
                                  name=f"p1_{k}_{c0}")
                    pb = pp1.tile([128, CH], F32, tag="p1",
                                  name=f"p1_{k}_{c0 + 1}")
                    if k == 0:
                        nc.tensor.matmul(pa[:], wfu_t[:], fixedT[:, sl0],
                                         start=True, stop=True)
                        nc.tensor.matmul(pb[:], wfu_t[:], fixedT[:, sl1],
                                         start=True, stop=True)
                    else:
                        nc.tensor.matmul(pa[:], we1_fx, fixedT[:, sl0],
                                         start=True, stop=False)
                        nc.tensor.matmul(pb[:], we1_fx, fixedT[:, sl1],
                                         start=True, stop=False)
                        nc.tensor.matmul(pa[:], we1_up, u_prev[:, sl0],
                                         start=False, stop=True)
                        nc.tensor.matmul(pb[:], we1_up, u_prev[:, sl1],
                                         start=False, stop=True)
                    p1ring[c0] = pa
                    p1ring[c0 + 1] = pb

                p1ring = {}
                s1_pair(0)

                cls_state = {"nf": 0, "pend": []}

                def emit_cls(n):
                    if k == 0:
                        return
                    fronts = []
                    while n > 0 and cls_state["nf"] < NCH:
                        c = cls_state["nf"]
                        fronts.append((c, cls_front(u_prev, k - 1, c)))
                        cls_state["nf"] = c + 1
                        n -= 1
                    cls_state["pend"].extend(fronts)
                    while len(cls_state["pend"]) > 1:
                        c, hcp = cls_state["pend"].pop(0)
                        cls_back(k - 1, c, hcp)

                def finish_cls():
                    while cls_state["pend"]:
                        c, hcp = cls_state["pend"].pop(0)
                        cls_back(k - 1, c, hcp)

                # classifier cover: all 16 chunks for k==1 (the first AR
                # is cold and needs a much bigger shadow), else 6
                NCLS0 = NCH if k == 1 else 8
                emit_cls(NCLS0)

                if k > 0:
                    # te(k) = mlp2([te(k-1), msga(k-1)])
                    pt1 = pp3.tile([128, AL], F32, tag="p3")
                    nc.tensor.matmul(pt1[:], wn1_a, te_prev[:],
                                     start=True, stop=False)
                    nc.tensor.matmul(pt1[:], wn1_b, msga[(k + 1) % 2][:],
                                     start=False, stop=True)
                    tn1 = wk.tile([128, AL], F32R, tag="tn1", bufs=2)
                    nc.scalar.activation(tn1[:], pt1[:], AF.Relu,
                                         bias=biases["bn1"][:, 0:1])
                    pt2 = pp3.tile([128, AL], F32, tag="p3")
                    nc.tensor.matmul(pt2[:], wn2_t[:], tn1[:],
                                     start=True, stop=True)
                    nc.scalar.activation(te_cur[:], pt2[:], AF.Relu,
                                         bias=biases["bn2"][:, 0:1])

                # Tb = W1a @ te(k) + be1  (h1 bias, (128, AL))
                pT = pp3.tile([128, AL], F32, tag="p3")
                nc.tensor.matmul(pT[:], we1_te, te_cur[:],
                                 start=True, stop=True)
                Tb = wk.tile([128, AL], F32, tag="tb", bufs=2)
                nc.scalar.activation(Tb[:], pT[:], AF.Identity,
                                     bias=biases["be1"][:, 0:1])

                # ===== AR(k-1)-dependent section =====
                # The ce-update chain alternates PE and scalar; classifier
                # chunks 6-8 are interleaved so the in-order PE queue has
                # independent work during the scalar hops.
                Cs = wk.tile([128, B], F32R, name=f"Cs{k}", tag="csb",
                             bufs=2)
                if k > 0:
                    mb_out_p, = arbufs.pop("out")
                    msgb_in = wk.tile([128, 2 * B], F32, tag="mbf", bufs=2)
                    nc.sync.dma_start(msgb_in[:, 0:B], mb_out_p[0:128, :])
                    nc.scalar.dma_start(msgb_in[:, B:2 * B],
                                        mb_out_p[128:256, :])
                    pc1 = pp3.tile([128, B], F32, tag="p3")
                    nc.tensor.matmul(pc1[:], wn1_a, ce_prev[:],
                                     start=True, stop=False)
                    nc.tensor.matmul(pc1[:], wn1_b,
                                     msgb_in[:, 0:B].bitcast(F32R),
                                     start=False, stop=False)
                    nc.tensor.matmul(pc1[:], wn1_b,
                                     msgb_in[:, B:2 * B].bitcast(F32R),
                                     start=False, stop=True)
                    cn1 = wk.tile([128, B], F32R, tag="cn1", bufs=2)
                    nc.scalar.activation(cn1[:], pc1[:], AF.Relu,
                                         bias=biases["bn1"][:, 0:1])
                    emit_cls(1)
                    pc2 = pp3.tile([128, B], F32, tag="p3")
                    nc.tensor.matmul(pc2[:], wn2_t[:], cn1[:],
                                     start=True, stop=True)
                    nc.scalar.activation(ce_cur[:], pc2[:], AF.Relu,
                                         bias=biases["bn2"][:, 0:1])
                    emit_cls(1)
                    pC = pp3.tile([128, B], F32, tag="p3")
                    nc.tensor.matmul(pC[:], we1_ce, ce_cur[:],
                                     start=True, stop=True)
                    nc.vector.tensor_copy(Cs[:], pC[:])
                    emit_cls(1)
                else:
                    pC = pp3.tile([128, B], F32, tag="p3")
                    nc.tensor.matmul(pC[:], we1_ce, ce_cur[:],
                                     start=True, stop=True)
                    nc.vector.tensor_copy(Cs[:], pC[:])

                accs = {}

                def do_pair(c):
                    g0 = u_cur[:, CH * c:CH * c + B].bitcast(F32)
                    g1 = u_cur[:, CH * c + B:CH * (c + 1)].bitcast(F32)
                    if c < 2:
                        acc = wk.tile([128, B], F32, tag="acc", bufs=4,
                                      name=f"acc{k}_{c}")
                        nc.vector.tensor_add(acc[:], g0, g1)
                        accs[c] = acc
                    else:
                        pr = wk.tile([128, B], F32, tag="pair", bufs=6,
                                     name=f"pr{k}_{c}")
                        nc.vector.tensor_add(pr[:], g0, g1)
                        nc.vector.tensor_add(accs[c % 2][:], accs[c % 2][:],
                                             pr[:])

                def msga_part(q):
                    # per-a sums over b for chunks 4q..4q+3 (contiguous)
                    seg = u_cur[:, 2048 * q:2048 * (q + 1)].bitcast(F32)
                    nc.vector.tensor_reduce(
                        msga_f[:, 8 * q:8 * (q + 1)],
                        seg.rearrange("p (a b) -> p a b", a=8),
                        mybir.AxisListType.X, ALU.add)

                if not last:
                    msga_f = wk.tile([128, AL], F32, tag="msgaf", bufs=2,
                                     name=f"msgaf{k}")

                for cp in range(0, NCH, 2):
                    ts = []
                    for c in (cp, cp + 1):
                        p1 = p1ring.pop(c)
                        t = wk.tile([128, CH], F32R, tag="t", bufs=4,
                                    name=f"t{k}_{c}")
                        for g in range(2):
                            bsl = slice(B * g, B * (g + 1))
                            col = 2 * c + g
                            nc.vector.scalar_tensor_tensor(
                                t[:, bsl], p1[:, bsl], Tb[:, col:col + 1],
                                Cs[:], ALU.add, ALU.add)
                        nc.scalar.activation(t[:], t[:], AF.Relu)
                        ts.append(t)
                    p2s = []
                    for t in ts:
                        p2 = pp2.tile([128, CH], F32, tag="p2")
                        nc.tensor.matmul(p2[:], we2_t[:], t[:],
                                         start=True, stop=True)
                        p2s.append(p2)
                    for i, c in enumerate((cp, cp + 1)):
                        sl = slice(CH * c, CH * (c + 1))
                        if last or c < 12:
                            nc.scalar.activation(u_cur[:, sl], p2s[i][:],
                                                 AF.Relu,
                                                 bias=biases["be2"][:, 0:1])
                        else:
                            # accum_out fills msga directly for the last 4
                            # chunks, keeping the step tail off the vector
                            # queue (the strided reduce there gated the AR
                            # trigger and the next step's te-update)
                            for g in range(2):
                                gsl = slice(CH * c + B * g,
                                            CH * c + B * (g + 1))
                                col = 2 * c + g
                                with nc.allow_low_precision(
                                        reason="f32r accum is 32-bit"):
                                    nc.scalar.activation(
                                        u_cur[:, gsl],
                                        p2s[i][:, B * g:B * (g + 1)],
                                        AF.Relu,
                                        bias=biases["be2"][:, 0:1],
                                        accum_out=msga[k % 2][:,
                                                           col:col + 1])
                    if cp + 2 < NCH:
                        s1_pair(cp + 2)
                    emit_cls(2)
                    if not last and cp >= 2:
                        do_pair(cp - 2)
                        do_pair(cp - 1)
                        if cp in (6, 10, 14):
                            msga_part(cp // 4 - 1)
                        if cp == 14:
                            nc.vector.tensor_copy(msga[k % 2][:, 0:24],
                                                  msga_f[:, 0:24])
                finish_cls()

                if not last:
                    do_pair(NCH - 2)
                    do_pair(NCH - 1)
                    # the two accumulator halves are summed by the CCE in
                    # the AllReduce itself (payload is latency-flat) and
                    # by the ce-update's accumulating matmuls afterwards
                    mb_in = dr.tile([2 * 128, B], F32, tag="mbin", bufs=2)
                    mb_out = dr.tile([2 * 128, B], F32, tag="mbout", bufs=2,
                                     addr_space="Shared")
                    nc.sync.dma_start(mb_in[0:128, :], accs[0][:])
                    nc.scalar.dma_start(mb_in[128:256, :], accs[1][:])
                    nc.gpsimd.collective_compute(
                        "AllReduce", mybir.AluOpType.add,
                        replica_groups=[list(range(8))],
                        ins=[mb_in.opt()], outs=[mb_out.opt()])
                    arbufs["out"] = (mb_out,)

            # final classifier for step 7
            u7 = upds[(STEPS - 1) % 2]
            hc_pend = None
            for c in range(NCH):
                hc_new = cls_front(u7, STEPS - 1, c)
                if hc_pend is not None:
                    cls_back(STEPS - 1, c - 1, hc_pend)
                hc_pend = hc_new
            cls_back(STEPS - 1, NCH - 1, hc_pend)

    nc.finalize()
    _BUILD_CACHE["nc"] = nc
    return nc


def _make_in_maps(inputs):
    f32 = np.float32

    def c(x):
        return np.ascontiguousarray(np.asarray(x, dtype=f32))

    tf = c(inputs["track_features"])
    cf = c(inputs["current_features"])
    tb = c(inputs["track_boxes"])
    cb = c(inputs["current_boxes"])
    tt = c(inputs["track_time"]).reshape(-1, 1)
    ct = c(inputs["current_time"]).reshape(-1, 1)

    shared = {
        "cfT": c(cf.T),
        "curf": cf,
        "curg": c(np.concatenate([cb, ct], axis=1)),
        "wlinT": c(inputs["w_lin"].T),
        "blin": c(np.broadcast_to(inputs["b_lin"][:, None], (D, 1))),
        "wein1T": c(inputs["w_ein1"].T),
        "bein1": c(inputs["b_ein1"][:, None]),
        "wein2T": c(inputs["w_ein2"].T),
        "bein2": c(inputs["b_ein2"][:, None]),
        "we1T": c(inputs["w_e1"].T),
        "be1": c(inputs["b_e1"][:, None]),
        "we2T": c(inputs["w_e2"].T),
        "be2": c(inputs["b_e2"][:, None]),
        "wn1T": c(inputs["w_n1"].T),
        "bn1": c(inputs["b_n1"][:, None]),
        "wn2T": c(inputs["w_n2"].T),
        "bn2": c(inputs["b_n2"][:, None]),
        "wc1T": c(inputs["w_c1"].T),
        "bc1": c(inputs["b_c1"][:, None]),
        "wc2c": c(inputs["w_c2"].T),
    }
    in_maps = []
    for core in range(8):
        rows = slice(AL * core, AL * (core + 1))
        m = dict(shared)
        m["tfT"] = c(tf[rows].T)
        m["trkf"] = c(tf[rows])
        m["trkg"] = c(np.concatenate([tb[rows], tt[rows]], axis=1))
        in_maps.append(m)
    return in_maps


def run(trace=False, trace_cores=None, **inputs):
    from concourse.bass_utils import run_bass_kernel_spmd

    if trace:
        _install_ntff_hook()
    nc = _build()
    in_maps = _make_in_maps(inputs)
    res = run_bass_kernel_spmd(nc, in_maps, core_ids=list(range(8)),
                               trace=trace, trace_cores=trace_cores)
    full = np.empty((STEPS, A, B), np.float32)
    for core in range(8):
        full[:, AL * core:AL * (core + 1), :] = \
            res.results[core]["out"].reshape(STEPS, AL, B)
    full += np.asarray(inputs["b_c2"], np.float32).reshape(1, 1, 1)
    return full, res


def kernel(**inputs):
    full, _ = run(trace=False, **inputs)
    return full


def _install_ntff_hook():
    import sys
    import types
    try:
        from antenv.axon_hooks import get_axon_ntff_profile_hook  # noqa: F401
        return
    except ImportError:
        pass
    import antenv
    from trn_agent_boot.trn_boot import _ntff_profile_via_ctypes

    mod = types.ModuleType("antenv.axon_hooks")
    holder = [_ntff_profile_via_ctypes("/opt/axon/libaxon_pjrt.so")]
    mod.get_axon_ntff_profile_hook = lambda: holder[0]
    mod.set_axon_ntff_profile_hook = lambda h: holder.__setitem__(0, h)
    sys.modules["antenv.axon_hooks"] = mod
    antenv.axon_hooks = mod


# revision 30
# speedup vs baseline: 1.1354x; 1.0501x over previous
"""AssignmentSimilarityNet GNN message-passing kernel for 8 Trainium2
NeuronCores.

Sharding: track (A) dimension split across 8 cores (32 tracks each).
Edge tensors, track embeds and messages-to-A stay local; messages-to-B
(sum over A) are all-reduced each step; MLP weights replicated.

Schedule (per step, steady state) — built around the ~23us serial
latency of one 8-core AllReduce (collectives cannot be pipelined in
the CC engine, so exactly one AR per step, fully covered):

  cover section (AR(k-1) in flight):
    - te(k) node update (local, from msga(k-1)); Tb = W1a@te + be1
    - S1 prologue: first two chunks' fixed+upd K-tile matmuls into PSUM
    - classifier chunks 0..5 of step k-1, software-pipelined on the PE
      (wc1 of chunk c+1 issues before the M=1 wc2 of chunk c); logits
      land in (1,512) PSUM rows, staged to SBUF (vector/scalar
      alternating), one small DMA per chunk
  AR-dependent section:
    - fetch msgb (split across sync+scalar queues), ce(k) node update
      with classifier chunks 6..8 interleaved into the PE queue so the
      in-order queue has work during the scalar hops; C = W1b @ ce(k)
    - per chunk: t = (p1 + Tb_col) + C via two scalar_tensor_tensor ops
      (vector), relu in place (scalar), we2 matmul, u = relu(p2+be2)
      (scalar); remaining S1 and classifier chunks interleaved
    - msgb via contiguous pair adds + two accumulator chains (vector);
      msga via 4 contiguous strided-view reduces off the critical path;
      stage DMA, trigger AR(k) (gpsimd queue holds only the triggers)
  The gpsimd/Pool engine is useless for bulk elementwise work (~9us for
  a 512-wide relu) and cannot touch PSUM; it only issues the collective
  triggers so no compute queue ever blocks on an AR.

The 8th AR is skipped (last node update is dead).  b_c2 is added on the
host.  All compute is fp32/fp32r: bf16 anywhere in the recurrent or
message path overshoots the 2e-2 error budget (measured 2.6e-2 for a
bf16 msgb alone; the net amplifies per-op rounding ~100x over 8 steps).
"""
import numpy as np

A = 256          # tracks
B = 256          # current detections
AL = A // 8      # tracks per core (32)
REID = 512
D = 128          # ND == ED
STEPS = 8
NP = AL * B      # pairs per core (8192)
CH = 512         # pair chunk (2 a-groups x 256 b)
NCH = NP // CH   # 16

_BUILD_CACHE = {}


def _build():
    if "nc" in _BUILD_CACHE:
        return _BUILD_CACHE["nc"]
    import concourse.bacc as bacc
    import concourse.mybir as mybir
    import concourse.tile as tile

    F32 = mybir.dt.float32
    F32R = mybir.dt.float32r
    AF = mybir.ActivationFunctionType
    ALU = mybir.AluOpType

    nc = bacc.Bacc(None, target_bir_lowering=False)

    def din(name, shape):
        return nc.dram_tensor(name, shape, F32, kind="ExternalInput")

    tfT = din("tfT", [REID, AL])
    trkf = din("trkf", [AL, REID])
    cfT = din("cfT", [REID, B])
    curf = din("curf", [B, REID])
    trkg = din("trkg", [AL, 5])
    curg = din("curg", [B, 5])
    wlinT = din("wlinT", [REID, D])
    blin = din("blin", [D, 1])
    wein1T = din("wein1T", [6, D])
    bein1 = din("bein1", [D, 1])
    wein2T = din("wein2T", [D, D])
    bein2 = din("bein2", [D, 1])
    we1T = din("we1T", [4 * D, D])
    be1 = din("be1", [D, 1])
    we2T = din("we2T", [D, D])
    be2 = din("be2", [D, 1])
    wn1T = din("wn1T", [2 * D, D])
    bn1 = din("bn1", [D, 1])
    wn2T = din("wn2T", [D, D])
    bn2 = din("bn2", [D, 1])
    wc1T = din("wc1T", [D, D])
    bc1 = din("bc1", [D, 1])
    wc2c = din("wc2c", [D, 1])
    out = nc.dram_tensor("out", [STEPS, NP], F32, kind="ExternalOutput")

    with tile.TileContext(nc) as tc:
        with (
            tc.tile_pool(name="const", bufs=1) as cp,
            tc.tile_pool(name="state", bufs=1) as st,
            tc.tile_pool(name="work", bufs=1) as wk,
            tc.tile_pool(name="p1", bufs=2, space="PSUM") as pp1,
            tc.tile_pool(name="p2", bufs=2, space="PSUM") as pp2,
            tc.tile_pool(name="p3", bufs=2, space="PSUM") as pp3,
            tc.tile_pool(name="plg", bufs=2, space="PSUM") as plgp,
            tc.tile_pool(name="dram", bufs=1, space="DRAM") as dr,
        ):
            # ---------------- feature loads ----------------
            tf_t = st.tile([128, 4 * AL], F32R)       # 4 K-tiles of (128, 32)
            cf_t = st.tile([128, 4 * B], F32R)        # 4 K-tiles of (128, 256)
            tf_s = wk.tile([128, 4 * AL], F32, tag="wstage", bufs=2)
            cf_s = wk.tile([128, 4 * B], F32, tag="wstage", bufs=2)
            for j in range(4):
                nc.sync.dma_start(tf_s[:, AL * j:AL * (j + 1)],
                                  tfT[128 * j:128 * (j + 1), :])
                nc.sync.dma_start(cf_s[:, B * j:B * (j + 1)],
                                  cfT[128 * j:128 * (j + 1), :])
            for j in range(4):
                nc.vector.tensor_copy(tf_t[:, AL * j:AL * (j + 1)],
                                      tf_s[:, AL * j:AL * (j + 1)])
                nc.vector.tensor_copy(cf_t[:, B * j:B * (j + 1)],
                                      cf_s[:, B * j:B * (j + 1)])
            trkf_t = wk.tile([AL, REID], F32)
            nc.scalar.dma_start(trkf_t[:], trkf[:])
            curf_t0 = wk.tile([128, REID], F32)
            curf_t1 = wk.tile([128, REID], F32)
            nc.scalar.dma_start(curf_t0[:], curf[0:128, :])
            nc.scalar.dma_start(curf_t1[:], curf[128:256, :])
            trkg_t = wk.tile([AL, 5], F32)
            nc.scalar.dma_start(trkg_t[:], trkg[:])
            curg_t0 = wk.tile([128, 5], F32)
            curg_t1 = wk.tile([128, 5], F32)
            nc.scalar.dma_start(curg_t0[:], curg[0:128, :])
            nc.scalar.dma_start(curg_t1[:], curg[128:256, :])

            # ---------------- reid norms ----------------
            sq_t = wk.tile([AL, REID], F32, tag="sq", bufs=1)
            nc.vector.tensor_mul(sq_t[:], trkf_t[:], trkf_t[:])
            sst = wk.tile([AL, 1], F32)
            nc.vector.tensor_reduce(sst[:], sq_t[:], mybir.AxisListType.X, ALU.add)
            rt = wk.tile([AL, 1], F32)
            nc.vector.reciprocal(rt[:], sst[:])
            inv_t = wk.tile([AL, 1], F32)
            nc.scalar.activation(inv_t[:], rt[:], AF.Sqrt)

            invc = []
            for i, ct in enumerate((curf_t0, curf_t1)):
                sq_c = wk.tile([128, REID], F32, name=f"sq_c{i}", tag="sq", bufs=1)
                nc.vector.tensor_mul(sq_c[:], ct[:], ct[:])
                ssc = wk.tile([128, 1], F32, name=f"ssc{i}")
                nc.vector.tensor_reduce(ssc[:], sq_c[:], mybir.AxisListType.X,
                                        ALU.add)
                rc = wk.tile([128, 1], F32, name=f"rc{i}")
                nc.vector.reciprocal(rc[:], ssc[:])
                ic = wk.tile([128, 1], F32, name=f"ic{i}")
                nc.scalar.activation(ic[:], rc[:], AF.Sqrt)
                invc.append(ic)

            # ---------------- current-side geometry -> bcast rows ----------
            # rows of cstage: 0 xb, 1 yb, 2 hb, 3 ln hb, 4 ln wb, 5 tb, 6 invc
            cstage = dr.tile([7, B], F32)
            for i, gt in enumerate((curg_t0, curg_t1)):
                half = slice(128 * i, 128 * (i + 1))
                cg = wk.tile([128, 7], F32, name=f"cg{i}")
                nc.vector.tensor_add(cg[:, 0:1], gt[:, 0:1], gt[:, 2:3])
                nc.vector.tensor_scalar_mul(cg[:, 0:1], cg[:, 0:1], 0.5)
                nc.vector.tensor_add(cg[:, 1:2], gt[:, 1:2], gt[:, 3:4])
                nc.vector.tensor_scalar_mul(cg[:, 1:2], cg[:, 1:2], 0.5)
                nc.vector.tensor_sub(cg[:, 2:3], gt[:, 3:4], gt[:, 1:2])
                wb = wk.tile([128, 1], F32, name=f"wb{i}")
                nc.vector.tensor_sub(wb[:], gt[:, 2:3], gt[:, 0:1])
                nc.scalar.activation(cg[:, 3:4], cg[:, 2:3], AF.Ln)
                nc.scalar.activation(cg[:, 4:5], wb[:], AF.Ln)
                nc.vector.tensor_copy(cg[:, 5:6], gt[:, 4:5])
                nc.vector.tensor_copy(cg[:, 6:7], invc[i][:])
                nc.sync.dma_start(cstage[:, half].transpose((1, 0)), cg[:])
            bcall = wk.tile([AL, 7 * B], F32)
            nc.sync.dma_start(
                bcall[:], cstage[:, :].partition_broadcast(AL)
                .rearrange("p r b -> p (r b)"))
            bc = {nm: bcall[:, B * r:B * (r + 1)]
                  for r, nm in enumerate(["xb", "yb", "hb", "lnhb",
                                          "lnwb", "tb", "invc"])}

            # ---------------- track-side geometry scalars ----------------
            xt = wk.tile([AL, 1], F32)
            nc.vector.tensor_add(xt[:], trkg_t[:, 0:1], trkg_t[:, 2:3])
            nc.vector.tensor_scalar_mul(xt[:], xt[:], 0.5)
            yt = wk.tile([AL, 1], F32)
            nc.vector.tensor_add(yt[:], trkg_t[:, 1:2], trkg_t[:, 3:4])
            nc.vector.tensor_scalar_mul(yt[:], yt[:], 0.5)
            ht = wk.tile([AL, 1], F32)
            nc.vector.tensor_sub(ht[:], trkg_t[:, 3:4], trkg_t[:, 1:2])
            wt = wk.tile([AL, 1], F32)
            nc.vector.tensor_sub(wt[:], trkg_t[:, 2:3], trkg_t[:, 0:1])
            lnht = wk.tile([AL, 1], F32)
            nc.scalar.activation(lnht[:], ht[:], AF.Ln)
            lnwt = wk.tile([AL, 1], F32)
            nc.scalar.activation(lnwt[:], wt[:], AF.Ln)

            # ---------------- edge features (AL, B) each ----------------
            den = wk.tile([AL, B], F32)
            nc.vector.tensor_scalar_add(den[:], bc["hb"][:], ht[:, 0:1])
            rden = wk.tile([AL, B], F32)
            nc.vector.reciprocal(rden[:], den[:])

            feats = []
            f0 = wk.tile([AL, B], F32, name="f_x")
            nc.vector.tensor_scalar(f0[:], bc["xb"][:], xt[:, 0:1], 2.0,
                                    ALU.subtract, ALU.mult)
            nc.vector.tensor_mul(f0[:], f0[:], rden[:])
            feats.append(f0)
            f1 = wk.tile([AL, B], F32, name="f_y")
            nc.vector.tensor_scalar(f1[:], bc["yb"][:], yt[:, 0:1], 2.0,
                                    ALU.subtract, ALU.mult)
            nc.vector.tensor_mul(f1[:], f1[:], rden[:])
            feats.append(f1)
            f2 = wk.tile([AL, B], F32, name="f_w")
            nc.vector.tensor_scalar(f2[:], bc["lnwb"][:], -1.0, lnwt[:, 0:1],
                                    ALU.mult, ALU.add)
            feats.append(f2)
            f3 = wk.tile([AL, B], F32, name="f_h")
            nc.vector.tensor_scalar(f3[:], bc["lnhb"][:], -1.0, lnht[:, 0:1],
                                    ALU.mult, ALU.add)
            feats.append(f3)
            f4 = wk.tile([AL, B], F32, name="f_t")
            nc.vector.tensor_scalar_sub(f4[:], bc["tb"][:], trkg_t[:, 4:5])
            feats.append(f4)

            # dist_reid = 1 - gram * inv_t * inv_c
            pg = pp3.tile([AL, B], F32, tag="p3")
            for j in range(4):
                nc.tensor.matmul(pg[:], tf_t[:, AL * j:AL * (j + 1)],
                                 cf_t[:, B * j:B * (j + 1)],
                                 start=(j == 0), stop=(j == 3))
            f5 = wk.tile([AL, B], F32, name="f_d")
            nc.vector.tensor_scalar(f5[:], pg[:], inv_t[:, 0:1], None,
                                    ALU.mult)
            nc.vector.tensor_mul(f5[:], f5[:], bc["invc"][:])
            nc.scalar.activation(f5[:], f5[:], AF.Copy, bias=1.0, scale=-1.0)
            feats.append(f5)

            # ---------------- transpose features -> efT (6, 8192) ----------
            ef_stage = dr.tile([6, NP], F32R)
            for f, t in enumerate(feats):
                fr = wk.tile([AL, B], F32R, name=f"fr{f}", tag="fr",
                             bufs=2)
                nc.vector.tensor_copy(fr[:], t[:])
                nc.sync.dma_start(
                    ef_stage[f:f + 1, :].rearrange("o (a b) -> (o a) b", a=AL),
                    fr[:])
            upds = [st.tile([128, NP], F32R, name="updA"),
                    st.tile([128, NP], F32R, name="updB")]
            efT_t = upds[0][0:6, :]
            nc.sync.dma_start(efT_t, ef_stage[:])

            # ---------------- weight / bias loads ----------------
            we1_t = cp.tile([128, 4 * D], F32R)
            wlin_t = cp.tile([128, 4 * D], F32R)
            wn1_t = cp.tile([128, 2 * D], F32R)
            we1_s = wk.tile([128, 4 * D], F32, tag="wstage", bufs=2)
            wlin_s = wk.tile([128, 4 * D], F32, tag="wstage", bufs=2)
            wn1_s = wk.tile([128, 2 * D], F32, tag="wstage", bufs=2)
            for j in range(4):
                nc.scalar.dma_start(we1_s[:, 128 * j:128 * (j + 1)],
                                    we1T[128 * j:128 * (j + 1), :])
                nc.scalar.dma_start(wlin_s[:, 128 * j:128 * (j + 1)],
                                    wlinT[128 * j:128 * (j + 1), :])
            for j in range(2):
                nc.scalar.dma_start(wn1_s[:, 128 * j:128 * (j + 1)],
                                    wn1T[128 * j:128 * (j + 1), :])
            nc.vector.tensor_copy(we1_t[:], we1_s[:])
            nc.vector.tensor_copy(wlin_t[:], wlin_s[:])
            nc.vector.tensor_copy(wn1_t[:], wn1_s[:])
            wein1_t = cp.tile([6, D], F32R)
            wein2_t = cp.tile([128, D], F32R)
            we2_t = cp.tile([128, D], F32R)
            wn2_t = cp.tile([128, D], F32R)
            wc1_t = cp.tile([128, D], F32R)
            wc2c_t = cp.tile([128, 1], F32R)
            for dst, src in [(wein1_t, wein1T), (wein2_t, wein2T),
                             (we2_t, we2T), (wn2_t, wn2T), (wc1_t, wc1T),
                             (wc2c_t, wc2c)]:
                s = wk.tile(list(dst.shape), F32, name=f"ws_{src.name}",
                            tag="wstage", bufs=2)
                nc.scalar.dma_start(s[:], src[:])
                nc.vector.tensor_copy(dst[:], s[:])
            # combined fixed+upd K-tile for step 0 (upd == fixed there)
            wfu_t = cp.tile([128, D], F32R)
            nc.vector.tensor_add(wfu_t[:], we1_t[:, 2 * D:3 * D],
                                 we1_t[:, 3 * D:4 * D])
            biases = {}
            for nm, src in [("blin", blin), ("bein1", bein1), ("bein2", bein2),
                            ("be1", be1), ("be2", be2), ("bn1", bn1),
                            ("bn2", bn2), ("bc1", bc1)]:
                t = cp.tile([128, 1], F32, name=f"b_{nm}")
                nc.scalar.dma_start(t[:], src[:])
                biases[nm] = t

            # ---------------- fixed_edge = mlp2(edge_feats) ----------------
            fixedT = st.tile([128, NP], F32R)
            for c in range(NCH):
                sl = slice(CH * c, CH * (c + 1))
                p1 = pp1.tile([128, CH], F32, tag="p1")
                nc.tensor.matmul(p1[:], wein1_t[:], efT_t[:, sl],
                                 start=True, stop=True)
                h = wk.tile([128, CH], F32R, tag="h1", bufs=2)
                if c % 2 == 0:
                    nc.scalar.activation(h[:], p1[:], AF.Relu,
                                         bias=biases["bein1"][:, 0:1])
                else:
                    nc.vector.tensor_scalar(h[:], p1[:],
                                            biases["bein1"][:, 0:1], 0.0,
                                            ALU.add, ALU.max)
                p2 = pp2.tile([128, CH], F32, tag="p2")
                nc.tensor.matmul(p2[:], wein2_t[:], h[:], start=True, stop=True)
                if c % 2 == 0:
                    nc.vector.tensor_scalar(fixedT[:, sl], p2[:],
                                            biases["bein2"][:, 0:1], 0.0,
                                            ALU.add, ALU.max)
                else:
                    nc.scalar.activation(fixedT[:, sl], p2[:], AF.Relu,
                                         bias=biases["bein2"][:, 0:1])

            # ---------------- initial node embeds ----------------
            pt = pp3.tile([128, AL], F32, tag="p3")
            for j in range(4):
                nc.tensor.matmul(pt[:], wlin_t[:, 128 * j:128 * (j + 1)],
                                 tf_t[:, AL * j:AL * (j + 1)],
                                 start=(j == 0), stop=(j == 3))
            te = [st.tile([128, AL], F32R, name="teA"),
                  st.tile([128, AL], F32R, name="teB")]
            nc.scalar.activation(te[0][:], pt[:], AF.Relu,
                                 bias=biases["blin"][:, 0:1])
            pc = pp3.tile([128, B], F32, tag="p3")
            for j in range(4):
                nc.tensor.matmul(pc[:], wlin_t[:, 128 * j:128 * (j + 1)],
                                 cf_t[:, B * j:B * (j + 1)],
                                 start=(j == 0), stop=(j == 3))
            ce = [st.tile([128, B], F32R, name="ceA"),
                  st.tile([128, B], F32R, name="ceB")]
            nc.scalar.activation(ce[0][:], pc[:], AF.Relu,
                                 bias=biases["blin"][:, 0:1])

            # ---------------- step-loop state tiles ----------------
            msga = [st.tile([128, AL], F32R, name="msgaA"),
                    st.tile([128, AL], F32R, name="msgaB")]
            we1_te = we1_t[:, 0:D]
            we1_ce = we1_t[:, D:2 * D]
            we1_up = we1_t[:, 2 * D:3 * D]
            we1_fx = we1_t[:, 3 * D:4 * D]
            wn1_a = wn1_t[:, 0:D]
            wn1_b = wn1_t[:, D:2 * D]

            def cls_front(u_src, kout, c):
                """wc1 + hc for classifier chunk c of step kout."""
                sl = slice(CH * c, CH * (c + 1))
                p3 = pp3.tile([128, CH], F32, tag="p3")
                nc.tensor.matmul(p3[:], wc1_t[:], u_src[:, sl],
                                 start=True, stop=True)
                hc = wk.tile([128, CH], F32R, tag="hc", bufs=4,
                             name=f"hc{kout}_{c}")
                if c % 2 == 0:
                    nc.scalar.activation(hc[:], p3[:], AF.Relu,
                                         bias=biases["bc1"][:, 0:1])
                else:
                    nc.vector.tensor_scalar(hc[:], p3[:],
                                            biases["bc1"][:, 0:1], 0.0,
                                            ALU.add, ALU.max)
                return hc

            def cls_back(kout, c, hc):
                """wc2 + logits staging + DMA for classifier chunk c."""
                plg = plgp.tile([1, CH], F32, tag="plg",
                                name=f"plg{kout}_{c}")
                nc.tensor.matmul(plg[:], wc2c_t[:], hc[:],
                                 start=True, stop=True)
                lg = wk.tile([1, CH], F32, tag="lg", bufs=6,
                             name=f"lg{kout}_{c}")
                if c % 4 == 0:
                    nc.vector.tensor_copy(lg[:], plg[:])
                else:
                    nc.scalar.activation(lg[:], plg[:], AF.Copy)
                nc.sync.dma_start(out[kout:kout + 1,
                                      CH * c:CH * (c + 1)], lg[:])

            arbufs = {}
            for k in range(STEPS):
                u_prev = fixedT if k == 0 else upds[(k + 1) % 2]
                u_cur = upds[k % 2]
                te_prev, te_cur = te[(k + 1) % 2], te[k % 2]
                ce_prev, ce_cur = ce[(k + 1) % 2], ce[k % 2]
                last = k == STEPS - 1

                def s1_pair(c0):
                    """S1 for chunks c0, c0+1 with same-weight matmuls
                    adjacent so LDWEIGHTS pre-loads and the second matmul
                    streams at full rate."""
                    sl0 = slice(CH * c0, CH * (c0 + 1))
                    sl1 = slice(CH * (c0 + 1), CH * (c0 + 2))
                    pa = pp1.tile([128, CH], F32, tag="p1",
                                  name=f"p1_{k}_{c0}")
                    pb = pp1.tile([128, CH], F32, tag="p1",
                                  name=f"p1_{k}_{c0 + 1}")
                    if k == 0:
                        nc.tensor.matmul(pa[:], wfu_t[:], fixedT[:, sl0],
                                         start=True, stop=True)
                        nc.tensor.matmul(pb[:], wfu_t[:], fixedT[:, sl1],
                                         start=True, stop=True)
                    else:
                        nc.tensor.matmul(pa[:], we1_fx, fixedT[:, sl0],
                                         start=True, stop=False)
                        nc.tensor.matmul(pb[:], we1_fx, fixedT[:, sl1],
                                         start=True, stop=False)
                        nc.tensor.matmul(pa[:], we1_up, u_prev[:, sl0],
                                         start=False, stop=True)
                        nc.tensor.matmul(pb[:], we1_up, u_prev[:, sl1],
                                         start=False, stop=True)
                    p1ring[c0] = pa
                    p1ring[c0 + 1] = pb

                p1ring = {}
                s1_pair(0)

                cls_state = {"nf": 0, "pend": []}

                def emit_cls(n):
                    if k == 0:
                        return
                    fronts = []
                    while n > 0 and cls_state["nf"] < NCH:
                        c = cls_state["nf"]
                        fronts.append((c, cls_front(u_prev, k - 1, c)))
                        cls_state["nf"] = c + 1
                        n -= 1
                    cls_state["pend"].extend(fronts)
                    while len(cls_state["pend"]) > 1:
                        c, hcp = cls_state["pend"].pop(0)
                        cls_back(k - 1, c, hcp)

                def finish_cls():
                    while cls_state["pend"]:
                        c, hcp = cls_state["pend"].pop(0)
                        cls_back(k - 1, c, hcp)

                # classifier cover: all 16 chunks for k==1 (the first AR
                # is cold and needs a much bigger shadow), else 6
                NCLS0 = NCH
                emit_cls(NCLS0)

                if k > 0:
                    # te(k) = mlp2([te(k-1), msga(k-1)])
                    pt1 = pp3.tile([128, AL], F32, tag="p3")
                    nc.tensor.matmul(pt1[:], wn1_a, te_prev[:],
                                     start=True, stop=False)
                    nc.tensor.matmul(pt1[:], wn1_b, msga[(k + 1) % 2][:],
                                     start=False, stop=True)
                    tn1 = wk.tile([128, AL], F32R, tag="tn1", bufs=2)
                    nc.scalar.activation(tn1[:], pt1[:], AF.Relu,
                                         bias=biases["bn1"][:, 0:1])
                    pt2 = pp3.tile([128, AL], F32, tag="p3")
                    nc.tensor.matmul(pt2[:], wn2_t[:], tn1[:],
                                     start=True, stop=True)
                    nc.scalar.activation(te_cur[:], pt2[:], AF.Relu,
                                         bias=biases["bn2"][:, 0:1])

                # Tb = W1a @ te(k) + be1  (h1 bias, (128, AL))
                pT = pp3.tile([128, AL], F32, tag="p3")
                nc.tensor.matmul(pT[:], we1_te, te_cur[:],
                                 start=True, stop=True)
                Tb = wk.tile([128, AL], F32, tag="tb", bufs=2)
                nc.scalar.activation(Tb[:], pT[:], AF.Identity,
                                     bias=biases["be1"][:, 0:1])

                # ===== AR(k-1)-dependent section =====
                # The ce-update chain alternates PE and scalar; classifier
                # chunks 6-8 are interleaved so the in-order PE queue has
                # independent work during the scalar hops.
                Cs = wk.tile([128, B], F32R, name=f"Cs{k}", tag="csb",
                             bufs=2)
                if k > 0:
                    mb_out_p, = arbufs.pop("out")
                    msgb_in = wk.tile([128, 2 * B], F32, tag="mbf", bufs=2)
                    nc.sync.dma_start(msgb_in[:, 0:B], mb_out_p[0:128, :])
                    nc.scalar.dma_start(msgb_in[:, B:2 * B],
                                        mb_out_p[128:256, :])
                    pc1 = pp3.tile([128, B], F32, tag="p3")
                    nc.tensor.matmul(pc1[:], wn1_a, ce_prev[:],
                                     start=True, stop=False)
                    nc.tensor.matmul(pc1[:], wn1_b,
                                     msgb_in[:, 0:B].bitcast(F32R),
                                     start=False, stop=False)
                    nc.tensor.matmul(pc1[:], wn1_b,
                                     msgb_in[:, B:2 * B].bitcast(F32R),
                                     start=False, stop=True)
                    cn1 = wk.tile([128, B], F32R, tag="cn1", bufs=2)
                    nc.scalar.activation(cn1[:], pc1[:], AF.Relu,
                                         bias=biases["bn1"][:, 0:1])
                    emit_cls(1)
                    pc2 = pp3.tile([128, B], F32, tag="p3")
                    nc.tensor.matmul(pc2[:], wn2_t[:], cn1[:],
                                     start=True, stop=True)
                    nc.scalar.activation(ce_cur[:], pc2[:], AF.Relu,
                                         bias=biases["bn2"][:, 0:1])
                    emit_cls(1)
                    pC = pp3.tile([128, B], F32, tag="p3")
                    nc.tensor.matmul(pC[:], we1_ce, ce_cur[:],
                                     start=True, stop=True)
                    nc.vector.tensor_copy(Cs[:], pC[:])
                    emit_cls(1)
                else:
                    pC = pp3.tile([128, B], F32, tag="p3")
                    nc.tensor.matmul(pC[:], we1_ce, ce_cur[:],
                                     start=True, stop=True)
                    nc.vector.tensor_copy(Cs[:], pC[:])

                accs = {}

                def do_pair(c):
                    g0 = u_cur[:, CH * c:CH * c + B].bitcast(F32)
                    g1 = u_cur[:, CH * c + B:CH * (c + 1)].bitcast(F32)
                    if c < 2:
                        acc = wk.tile([128, B], F32, tag="acc", bufs=4,
                                      name=f"acc{k}_{c}")
                        nc.vector.tensor_add(acc[:], g0, g1)
                        accs[c] = acc
                    else:
                        pr = wk.tile([128, B], F32, tag="pair", bufs=6,
                                     name=f"pr{k}_{c}")
                        nc.vector.tensor_add(pr[:], g0, g1)
                        nc.vector.tensor_add(accs[c % 2][:], accs[c % 2][:],
                                             pr[:])

                def msga_part(q):
                    # per-a sums over b for chunks 4q..4q+3 (contiguous)
                    seg = u_cur[:, 2048 * q:2048 * (q + 1)].bitcast(F32)
                    nc.vector.tensor_reduce(
                        msga_f[:, 8 * q:8 * (q + 1)],
                        seg.rearrange("p (a b) -> p a b", a=8),
                        mybir.AxisListType.X, ALU.add)

                if not last:
                    msga_f = wk.tile([128, AL], F32, tag="msgaf", bufs=2,
                                     name=f"msgaf{k}")

                for cp in range(0, NCH, 2):
                    ts = []
                    for c in (cp, cp + 1):
                        p1 = p1ring.pop(c)
                        t = wk.tile([128, CH], F32R, tag="t", bufs=4,
                                    name=f"t{k}_{c}")
                        for g in range(2):
                            bsl = slice(B * g, B * (g + 1))
                            col = 2 * c + g
                            nc.vector.scalar_tensor_tensor(
                                t[:, bsl], p1[:, bsl], Tb[:, col:col + 1],
                                Cs[:], ALU.add, ALU.add)
                        nc.scalar.activation(t[:], t[:], AF.Relu)
                        ts.append(t)
                    p2s = []
                    for t in ts:
                        p2 = pp2.tile([128, CH], F32, tag="p2")
                        nc.tensor.matmul(p2[:], we2_t[:], t[:],
                                         start=True, stop=True)
                        p2s.append(p2)
                    for i, c in enumerate((cp, cp + 1)):
                        sl = slice(CH * c, CH * (c + 1))
                        if last or c < 12:
                            nc.scalar.activation(u_cur[:, sl], p2s[i][:],
                                                 AF.Relu,
                                                 bias=biases["be2"][:, 0:1])
                        else:
                            # accum_out fills msga directly for the last 4
                            # chunks, keeping the step tail off the vector
                            # queue (the strided reduce there gated the AR
                            # trigger and the next step's te-update)
                            for g in range(2):
                                gsl = slice(CH * c + B * g,
                                            CH * c + B * (g + 1))
                                col = 2 * c + g
                                with nc.allow_low_precision(
                                        reason="f32r accum is 32-bit"):
                                    nc.scalar.activation(
                                        u_cur[:, gsl],
                                        p2s[i][:, B * g:B * (g + 1)],
                                        AF.Relu,
                                        bias=biases["be2"][:, 0:1],
                                        accum_out=msga[k % 2][:,
                                                           col:col + 1])
                    if cp + 2 < NCH:
                        s1_pair(cp + 2)
                    emit_cls(2)
                    if not last and cp >= 2:
                        do_pair(cp - 2)
                        do_pair(cp - 1)
                        if cp in (6, 10, 14):
                            msga_part(cp // 4 - 1)
                        if cp == 14:
                            nc.vector.tensor_copy(msga[k % 2][:, 0:24],
                                                  msga_f[:, 0:24])
                finish_cls()

                if not last:
                    do_pair(NCH - 2)
                    do_pair(NCH - 1)
                    # the two accumulator halves are summed by the CCE in
                    # the AllReduce itself (payload is latency-flat) and
                    # by the ce-update's accumulating matmuls afterwards
                    mb_in = dr.tile([2 * 128, B], F32, tag="mbin", bufs=2)
                    mb_out = dr.tile([2 * 128, B], F32, tag="mbout", bufs=2,
                                     addr_space="Shared")
                    nc.sync.dma_start(mb_in[0:128, :], accs[0][:])
                    nc.scalar.dma_start(mb_in[128:256, :], accs[1][:])
                    nc.gpsimd.collective_compute(
                        "AllReduce", mybir.AluOpType.add,
                        replica_groups=[list(range(8))],
                        ins=[mb_in.opt()], outs=[mb_out.opt()])
                    arbufs["out"] = (mb_out,)

            # final classifier for step 7
            u7 = upds[(STEPS - 1) % 2]
            hc_pend = None
            for c in range(NCH):
                hc_new = cls_front(u7, STEPS - 1, c)
                if hc_pend is not None:
                    cls_back(STEPS - 1, c - 1, hc_pend)
                hc_pend = hc_new
            cls_back(STEPS - 1, NCH - 1, hc_pend)

    nc.finalize()
    _BUILD_CACHE["nc"] = nc
    return nc


def _make_in_maps(inputs):
    f32 = np.float32

    def c(x):
        return np.ascontiguousarray(np.asarray(x, dtype=f32))

    tf = c(inputs["track_features"])
    cf = c(inputs["current_features"])
    tb = c(inputs["track_boxes"])
    cb = c(inputs["current_boxes"])
    tt = c(inputs["track_time"]).reshape(-1, 1)
    ct = c(inputs["current_time"]).reshape(-1, 1)

    shared = {
        "cfT": c(cf.T),
        "curf": cf,
        "curg": c(np.concatenate([cb, ct], axis=1)),
        "wlinT": c(inputs["w_lin"].T),
        "blin": c(np.broadcast_to(inputs["b_lin"][:, None], (D, 1))),
        "wein1T": c(inputs["w_ein1"].T),
        "bein1": c(inputs["b_ein1"][:, None]),
        "wein2T": c(inputs["w_ein2"].T),
        "bein2": c(inputs["b_ein2"][:, None]),
        "we1T": c(inputs["w_e1"].T),
        "be1": c(inputs["b_e1"][:, None]),
        "we2T": c(inputs["w_e2"].T),
        "be2": c(inputs["b_e2"][:, None]),
        "wn1T": c(inputs["w_n1"].T),
        "bn1": c(inputs["b_n1"][:, None]),
        "wn2T": c(inputs["w_n2"].T),
        "bn2": c(inputs["b_n2"][:, None]),
        "wc1T": c(inputs["w_c1"].T),
        "bc1": c(inputs["b_c1"][:, None]),
        "wc2c": c(inputs["w_c2"].T),
    }
    in_maps = []
    for core in range(8):
        rows = slice(AL * core, AL * (core + 1))
        m = dict(shared)
        m["tfT"] = c(tf[rows].T)
        m["trkf"] = c(tf[rows])
        m["trkg"] = c(np.concatenate([tb[rows], tt[rows]], axis=1))
        in_maps.append(m)
    return in_maps


def run(trace=False, trace_cores=None, **inputs):
    from concourse.bass_utils import run_bass_kernel_spmd

    if trace:
        _install_ntff_hook()
    nc = _build()
    in_maps = _make_in_maps(inputs)
    res = run_bass_kernel_spmd(nc, in_maps, core_ids=list(range(8)),
                               trace=trace, trace_cores=trace_cores)
    full = np.empty((STEPS, A, B), np.float32)
    for core in range(8):
        full[:, AL * core:AL * (core + 1), :] = \
            res.results[core]["out"].reshape(STEPS, AL, B)
    full += np.asarray(inputs["b_c2"], np.float32).reshape(1, 1, 1)
    return full, res


def kernel(**inputs):
    full, _ = run(trace=False, **inputs)
    return full


def _install_ntff_hook():
    import sys
    import types
    try:
        from antenv.axon_hooks import get_axon_ntff_profile_hook  # noqa: F401
        return
    except ImportError:
        pass
    import antenv
    from trn_agent_boot.trn_boot import _ntff_profile_via_ctypes

    mod = types.ModuleType("antenv.axon_hooks")
    holder = [_ntff_profile_via_ctypes("/opt/axon/libaxon_pjrt.so")]
    mod.get_axon_ntff_profile_hook = lambda: holder[0]
    mod.set_axon_ntff_profile_hook = lambda h: holder.__setitem__(0, h)
    sys.modules["antenv.axon_hooks"] = mod
    antenv.axon_hooks = mod


# revision 31
# speedup vs baseline: 1.1890x; 1.0473x over previous
"""AssignmentSimilarityNet GNN message-passing kernel for 8 Trainium2
NeuronCores.

Sharding: track (A) dimension split across 8 cores (32 tracks each).
Edge tensors, track embeds and messages-to-A stay local; messages-to-B
(sum over A) are all-reduced each step; MLP weights replicated.

Schedule (per step, steady state) — built around the ~23us serial
latency of one 8-core AllReduce (collectives cannot be pipelined in
the CC engine, so exactly one AR per step, fully covered):

  cover section (AR(k-1) in flight):
    - te(k) node update (local, from msga(k-1)); Tb = W1a@te + be1
    - S1 prologue: first two chunks' fixed+upd K-tile matmuls into PSUM
    - classifier chunks 0..5 of step k-1, software-pipelined on the PE
      (wc1 of chunk c+1 issues before the M=1 wc2 of chunk c); logits
      land in (1,512) PSUM rows, staged to SBUF (vector/scalar
      alternating), one small DMA per chunk
  AR-dependent section:
    - fetch msgb (split across sync+scalar queues), ce(k) node update
      with classifier chunks 6..8 interleaved into the PE queue so the
      in-order queue has work during the scalar hops; C = W1b @ ce(k)
    - per chunk: t = (p1 + Tb_col) + C via two scalar_tensor_tensor ops
      (vector), relu in place (scalar), we2 matmul, u = relu(p2+be2)
      (scalar); remaining S1 and classifier chunks interleaved
    - msgb via contiguous pair adds + two accumulator chains (vector);
      msga via 4 contiguous strided-view reduces off the critical path;
      stage DMA, trigger AR(k) (gpsimd queue holds only the triggers)
  The gpsimd/Pool engine is useless for bulk elementwise work (~9us for
  a 512-wide relu) and cannot touch PSUM; it only issues the collective
  triggers so no compute queue ever blocks on an AR.

The 8th AR is skipped (last node update is dead).  b_c2 is added on the
host.  All compute is fp32/fp32r: bf16 anywhere in the recurrent or
message path overshoots the 2e-2 error budget (measured 2.6e-2 for a
bf16 msgb alone; the net amplifies per-op rounding ~100x over 8 steps).
"""
import numpy as np

A = 256          # tracks
B = 256          # current detections
AL = A // 8      # tracks per core (32)
REID = 512
D = 128          # ND == ED
STEPS = 8
NP = AL * B      # pairs per core (8192)
CH = 512         # pair chunk (2 a-groups x 256 b)
NCH = NP // CH   # 16

_BUILD_CACHE = {}


def _build():
    if "nc" in _BUILD_CACHE:
        return _BUILD_CACHE["nc"]
    import concourse.bacc as bacc
    import concourse.mybir as mybir
    import concourse.tile as tile

    F32 = mybir.dt.float32
    F32R = mybir.dt.float32r
    AF = mybir.ActivationFunctionType
    ALU = mybir.AluOpType

    nc = bacc.Bacc(None, target_bir_lowering=False)

    def din(name, shape):
        return nc.dram_tensor(name, shape, F32, kind="ExternalInput")

    tfT = din("tfT", [REID, AL])
    trkf = din("trkf", [AL, REID])
    cfT = din("cfT", [REID, B])
    curf = din("curf", [B, REID])
    trkg = din("trkg", [AL, 5])
    curg = din("curg", [B, 5])
    wlinT = din("wlinT", [REID, D])
    blin = din("blin", [D, 1])
    wein1T = din("wein1T", [6, D])
    bein1 = din("bein1", [D, 1])
    wein2T = din("wein2T", [D, D])
    bein2 = din("bein2", [D, 1])
    we1T = din("we1T", [4 * D, D])
    be1 = din("be1", [D, 1])
    we2T = din("we2T", [D, D])
    be2 = din("be2", [D, 1])
    wn1T = din("wn1T", [2 * D, D])
    bn1 = din("bn1", [D, 1])
    wn2T = din("wn2T", [D, D])
    bn2 = din("bn2", [D, 1])
    wc1T = din("wc1T", [D, D])
    bc1 = din("bc1", [D, 1])
    wc2c = din("wc2c", [D, 1])
    out = nc.dram_tensor("out", [STEPS, NP], F32, kind="ExternalOutput")

    with tile.TileContext(nc) as tc:
        with (
            tc.tile_pool(name="const", bufs=1) as cp,
            tc.tile_pool(name="state", bufs=1) as st,
            tc.tile_pool(name="work", bufs=1) as wk,
            tc.tile_pool(name="p1", bufs=2, space="PSUM") as pp1,
            tc.tile_pool(name="p2", bufs=2, space="PSUM") as pp2,
            tc.tile_pool(name="p3", bufs=2, space="PSUM") as pp3,
            tc.tile_pool(name="plg", bufs=2, space="PSUM") as plgp,
            tc.tile_pool(name="dram", bufs=1, space="DRAM") as dr,
        ):
            # ---------------- feature loads ----------------
            tf_t = st.tile([128, 4 * AL], F32R)       # 4 K-tiles of (128, 32)
            cf_t = st.tile([128, 4 * B], F32R)        # 4 K-tiles of (128, 256)
            tf_s = wk.tile([128, 4 * AL], F32, tag="wstage", bufs=2)
            cf_s = wk.tile([128, 4 * B], F32, tag="wstage", bufs=2)
            for j in range(4):
                nc.sync.dma_start(tf_s[:, AL * j:AL * (j + 1)],
                                  tfT[128 * j:128 * (j + 1), :])
                nc.sync.dma_start(cf_s[:, B * j:B * (j + 1)],
                                  cfT[128 * j:128 * (j + 1), :])
            for j in range(4):
                nc.vector.tensor_copy(tf_t[:, AL * j:AL * (j + 1)],
                                      tf_s[:, AL * j:AL * (j + 1)])
                nc.vector.tensor_copy(cf_t[:, B * j:B * (j + 1)],
                                      cf_s[:, B * j:B * (j + 1)])
            trkf_t = wk.tile([AL, REID], F32)
            nc.scalar.dma_start(trkf_t[:], trkf[:])
            curf_t0 = wk.tile([128, REID], F32)
            curf_t1 = wk.tile([128, REID], F32)
            nc.scalar.dma_start(curf_t0[:], curf[0:128, :])
            nc.scalar.dma_start(curf_t1[:], curf[128:256, :])
            trkg_t = wk.tile([AL, 5], F32)
            nc.scalar.dma_start(trkg_t[:], trkg[:])
            curg_t0 = wk.tile([128, 5], F32)
            curg_t1 = wk.tile([128, 5], F32)
            nc.scalar.dma_start(curg_t0[:], curg[0:128, :])
            nc.scalar.dma_start(curg_t1[:], curg[128:256, :])

            # ---------------- reid norms ----------------
            sq_t = wk.tile([AL, REID], F32, tag="sq", bufs=1)
            nc.vector.tensor_mul(sq_t[:], trkf_t[:], trkf_t[:])
            sst = wk.tile([AL, 1], F32)
            nc.vector.tensor_reduce(sst[:], sq_t[:], mybir.AxisListType.X, ALU.add)
            rt = wk.tile([AL, 1], F32)
            nc.vector.reciprocal(rt[:], sst[:])
            inv_t = wk.tile([AL, 1], F32)
            nc.scalar.activation(inv_t[:], rt[:], AF.Sqrt)

            invc = []
            for i, ct in enumerate((curf_t0, curf_t1)):
                sq_c = wk.tile([128, REID], F32, name=f"sq_c{i}", tag="sq", bufs=1)
                nc.vector.tensor_mul(sq_c[:], ct[:], ct[:])
                ssc = wk.tile([128, 1], F32, name=f"ssc{i}")
                nc.vector.tensor_reduce(ssc[:], sq_c[:], mybir.AxisListType.X,
                                        ALU.add)
                rc = wk.tile([128, 1], F32, name=f"rc{i}")
                nc.vector.reciprocal(rc[:], ssc[:])
                ic = wk.tile([128, 1], F32, name=f"ic{i}")
                nc.scalar.activation(ic[:], rc[:], AF.Sqrt)
                invc.append(ic)

            # ---------------- current-side geometry -> bcast rows ----------
            # rows of cstage: 0 xb, 1 yb, 2 hb, 3 ln hb, 4 ln wb, 5 tb, 6 invc
            cstage = dr.tile([7, B], F32)
            for i, gt in enumerate((curg_t0, curg_t1)):
                half = slice(128 * i, 128 * (i + 1))
                cg = wk.tile([128, 7], F32, name=f"cg{i}")
                nc.vector.tensor_add(cg[:, 0:1], gt[:, 0:1], gt[:, 2:3])
                nc.vector.tensor_scalar_mul(cg[:, 0:1], cg[:, 0:1], 0.5)
                nc.vector.tensor_add(cg[:, 1:2], gt[:, 1:2], gt[:, 3:4])
                nc.vector.tensor_scalar_mul(cg[:, 1:2], cg[:, 1:2], 0.5)
                nc.vector.tensor_sub(cg[:, 2:3], gt[:, 3:4], gt[:, 1:2])
                wb = wk.tile([128, 1], F32, name=f"wb{i}")
                nc.vector.tensor_sub(wb[:], gt[:, 2:3], gt[:, 0:1])
                nc.scalar.activation(cg[:, 3:4], cg[:, 2:3], AF.Ln)
                nc.scalar.activation(cg[:, 4:5], wb[:], AF.Ln)
                nc.vector.tensor_copy(cg[:, 5:6], gt[:, 4:5])
                nc.vector.tensor_copy(cg[:, 6:7], invc[i][:])
                nc.sync.dma_start(cstage[:, half].transpose((1, 0)), cg[:])
            bcall = wk.tile([AL, 7 * B], F32)
            nc.sync.dma_start(
                bcall[:], cstage[:, :].partition_broadcast(AL)
                .rearrange("p r b -> p (r b)"))
            bc = {nm: bcall[:, B * r:B * (r + 1)]
                  for r, nm in enumerate(["xb", "yb", "hb", "lnhb",
                                          "lnwb", "tb", "invc"])}

            # ---------------- track-side geometry scalars ----------------
            xt = wk.tile([AL, 1], F32)
            nc.vector.tensor_add(xt[:], trkg_t[:, 0:1], trkg_t[:, 2:3])
            nc.vector.tensor_scalar_mul(xt[:], xt[:], 0.5)
            yt = wk.tile([AL, 1], F32)
            nc.vector.tensor_add(yt[:], trkg_t[:, 1:2], trkg_t[:, 3:4])
            nc.vector.tensor_scalar_mul(yt[:], yt[:], 0.5)
            ht = wk.tile([AL, 1], F32)
            nc.vector.tensor_sub(ht[:], trkg_t[:, 3:4], trkg_t[:, 1:2])
            wt = wk.tile([AL, 1], F32)
            nc.vector.tensor_sub(wt[:], trkg_t[:, 2:3], trkg_t[:, 0:1])
            lnht = wk.tile([AL, 1], F32)
            nc.scalar.activation(lnht[:], ht[:], AF.Ln)
            lnwt = wk.tile([AL, 1], F32)
            nc.scalar.activation(lnwt[:], wt[:], AF.Ln)

            # ---------------- edge features (AL, B) each ----------------
            den = wk.tile([AL, B], F32)
            nc.vector.tensor_scalar_add(den[:], bc["hb"][:], ht[:, 0:1])
            rden = wk.tile([AL, B], F32)
            nc.vector.reciprocal(rden[:], den[:])

            feats = []
            f0 = wk.tile([AL, B], F32, name="f_x")
            nc.vector.tensor_scalar(f0[:], bc["xb"][:], xt[:, 0:1], 2.0,
                                    ALU.subtract, ALU.mult)
            nc.vector.tensor_mul(f0[:], f0[:], rden[:])
            feats.append(f0)
            f1 = wk.tile([AL, B], F32, name="f_y")
            nc.vector.tensor_scalar(f1[:], bc["yb"][:], yt[:, 0:1], 2.0,
                                    ALU.subtract, ALU.mult)
            nc.vector.tensor_mul(f1[:], f1[:], rden[:])
            feats.append(f1)
            f2 = wk.tile([AL, B], F32, name="f_w")
            nc.vector.tensor_scalar(f2[:], bc["lnwb"][:], -1.0, lnwt[:, 0:1],
                                    ALU.mult, ALU.add)
            feats.append(f2)
            f3 = wk.tile([AL, B], F32, name="f_h")
            nc.vector.tensor_scalar(f3[:], bc["lnhb"][:], -1.0, lnht[:, 0:1],
                                    ALU.mult, ALU.add)
            feats.append(f3)
            f4 = wk.tile([AL, B], F32, name="f_t")
            nc.vector.tensor_scalar_sub(f4[:], bc["tb"][:], trkg_t[:, 4:5])
            feats.append(f4)

            # dist_reid = 1 - gram * inv_t * inv_c
            pg = pp3.tile([AL, B], F32, tag="p3")
            for j in range(4):
                nc.tensor.matmul(pg[:], tf_t[:, AL * j:AL * (j + 1)],
                                 cf_t[:, B * j:B * (j + 1)],
                                 start=(j == 0), stop=(j == 3))
            f5 = wk.tile([AL, B], F32, name="f_d")
            nc.vector.tensor_scalar(f5[:], pg[:], inv_t[:, 0:1], None,
                                    ALU.mult)
            nc.vector.tensor_mul(f5[:], f5[:], bc["invc"][:])
            nc.scalar.activation(f5[:], f5[:], AF.Copy, bias=1.0, scale=-1.0)
            feats.append(f5)

            # ---------------- transpose features -> efT (6, 8192) ----------
            ef_stage = dr.tile([6, NP], F32R)
            for f, t in enumerate(feats):
                fr = wk.tile([AL, B], F32R, name=f"fr{f}", tag="fr",
                             bufs=2)
                nc.vector.tensor_copy(fr[:], t[:])
                nc.sync.dma_start(
                    ef_stage[f:f + 1, :].rearrange("o (a b) -> (o a) b", a=AL),
                    fr[:])
            upds = [st.tile([128, NP], F32R, name="updA"),
                    st.tile([128, NP], F32R, name="updB")]
            efT_t = upds[0][0:6, :]
            nc.sync.dma_start(efT_t, ef_stage[:])

            # ---------------- weight / bias loads ----------------
            we1_t = cp.tile([128, 4 * D], F32R)
            wlin_t = cp.tile([128, 4 * D], F32R)
            wn1_t = cp.tile([128, 2 * D], F32R)
            we1_s = wk.tile([128, 4 * D], F32, tag="wstage", bufs=2)
            wlin_s = wk.tile([128, 4 * D], F32, tag="wstage", bufs=2)
            wn1_s = wk.tile([128, 2 * D], F32, tag="wstage", bufs=2)
            for j in range(4):
                nc.scalar.dma_start(we1_s[:, 128 * j:128 * (j + 1)],
                                    we1T[128 * j:128 * (j + 1), :])
                nc.scalar.dma_start(wlin_s[:, 128 * j:128 * (j + 1)],
                                    wlinT[128 * j:128 * (j + 1), :])
            for j in range(2):
                nc.scalar.dma_start(wn1_s[:, 128 * j:128 * (j + 1)],
                                    wn1T[128 * j:128 * (j + 1), :])
            nc.vector.tensor_copy(we1_t[:], we1_s[:])
            nc.vector.tensor_copy(wlin_t[:], wlin_s[:])
            nc.vector.tensor_copy(wn1_t[:], wn1_s[:])
            wein1_t = cp.tile([6, D], F32R)
            wein2_t = cp.tile([128, D], F32R)
            we2_t = cp.tile([128, D], F32R)
            wn2_t = cp.tile([128, D], F32R)
            wc1_t = cp.tile([128, D], F32R)
            wc2c_t = cp.tile([128, 1], F32R)
            for dst, src in [(wein1_t, wein1T), (wein2_t, wein2T),
                             (we2_t, we2T), (wn2_t, wn2T), (wc1_t, wc1T),
                             (wc2c_t, wc2c)]:
                s = wk.tile(list(dst.shape), F32, name=f"ws_{src.name}",
                            tag="wstage", bufs=2)
                nc.scalar.dma_start(s[:], src[:])
                nc.vector.tensor_copy(dst[:], s[:])
            # combined fixed+upd K-tile for step 0 (upd == fixed there)
            wfu_t = cp.tile([128, D], F32R)
            nc.vector.tensor_add(wfu_t[:], we1_t[:, 2 * D:3 * D],
                                 we1_t[:, 3 * D:4 * D])
            biases = {}
            for nm, src in [("blin", blin), ("bein1", bein1), ("bein2", bein2),
                            ("be1", be1), ("be2", be2), ("bn1", bn1),
                            ("bn2", bn2), ("bc1", bc1)]:
                t = cp.tile([128, 1], F32, name=f"b_{nm}")
                nc.scalar.dma_start(t[:], src[:])
                biases[nm] = t

            # ---------------- fixed_edge = mlp2(edge_feats) ----------------
            fixedT = st.tile([128, NP], F32R)
            for c in range(NCH):
                sl = slice(CH * c, CH * (c + 1))
                p1 = pp1.tile([128, CH], F32, tag="p1")
                nc.tensor.matmul(p1[:], wein1_t[:], efT_t[:, sl],
                                 start=True, stop=True)
                h = wk.tile([128, CH], F32R, tag="h1", bufs=2)
                if c % 2 == 0:
                    nc.scalar.activation(h[:], p1[:], AF.Relu,
                                         bias=biases["bein1"][:, 0:1])
                else:
                    nc.vector.tensor_scalar(h[:], p1[:],
                                            biases["bein1"][:, 0:1], 0.0,
                                            ALU.add, ALU.max)
                p2 = pp2.tile([128, CH], F32, tag="p2")
                nc.tensor.matmul(p2[:], wein2_t[:], h[:], start=True, stop=True)
                if c % 2 == 0:
                    nc.vector.tensor_scalar(fixedT[:, sl], p2[:],
                                            biases["bein2"][:, 0:1], 0.0,
                                            ALU.add, ALU.max)
                else:
                    nc.scalar.activation(fixedT[:, sl], p2[:], AF.Relu,
                                         bias=biases["bein2"][:, 0:1])

            # ---------------- initial node embeds ----------------
            pt = pp3.tile([128, AL], F32, tag="p3")
            for j in range(4):
                nc.tensor.matmul(pt[:], wlin_t[:, 128 * j:128 * (j + 1)],
                                 tf_t[:, AL * j:AL * (j + 1)],
                                 start=(j == 0), stop=(j == 3))
            te = [st.tile([128, AL], F32R, name="teA"),
                  st.tile([128, AL], F32R, name="teB")]
            nc.scalar.activation(te[0][:], pt[:], AF.Relu,
                                 bias=biases["blin"][:, 0:1])
            pc = pp3.tile([128, B], F32, tag="p3")
            for j in range(4):
                nc.tensor.matmul(pc[:], wlin_t[:, 128 * j:128 * (j + 1)],
                                 cf_t[:, B * j:B * (j + 1)],
                                 start=(j == 0), stop=(j == 3))
            ce = [st.tile([128, B], F32R, name="ceA"),
                  st.tile([128, B], F32R, name="ceB")]
            nc.scalar.activation(ce[0][:], pc[:], AF.Relu,
                                 bias=biases["blin"][:, 0:1])

            # ---------------- step-loop state tiles ----------------
            msga = [st.tile([128, AL], F32R, name="msgaA"),
                    st.tile([128, AL], F32R, name="msgaB")]
            we1_te = we1_t[:, 0:D]
            we1_ce = we1_t[:, D:2 * D]
            we1_up = we1_t[:, 2 * D:3 * D]
            we1_fx = we1_t[:, 3 * D:4 * D]
            wn1_a = wn1_t[:, 0:D]
            wn1_b = wn1_t[:, D:2 * D]

            def cls_front(u_src, kout, c):
                """wc1 + hc for classifier chunk c of step kout."""
                sl = slice(CH * c, CH * (c + 1))
                p3 = pp3.tile([128, CH], F32, tag="p3")
                nc.tensor.matmul(p3[:], wc1_t[:], u_src[:, sl],
                                 start=True, stop=True)
                hc = wk.tile([128, CH], F32R, tag="hc", bufs=4,
                             name=f"hc{kout}_{c}")
                if c % 2 == 0:
                    nc.scalar.activation(hc[:], p3[:], AF.Relu,
                                         bias=biases["bc1"][:, 0:1])
                else:
                    nc.vector.tensor_scalar(hc[:], p3[:],
                                            biases["bc1"][:, 0:1], 0.0,
                                            ALU.add, ALU.max)
                return hc

            def cls_back(kout, c, hc):
                """wc2 + logits staging + DMA for classifier chunk c."""
                plg = plgp.tile([1, CH], F32, tag="plg",
                                name=f"plg{kout}_{c}")
                nc.tensor.matmul(plg[:], wc2c_t[:], hc[:],
                                 start=True, stop=True)
                lg = wk.tile([1, CH], F32, tag="lg", bufs=6,
                             name=f"lg{kout}_{c}")
                if c % 4 == 0:
                    nc.vector.tensor_copy(lg[:], plg[:])
                else:
                    nc.scalar.activation(lg[:], plg[:], AF.Copy)
                nc.sync.dma_start(out[kout:kout + 1,
                                      CH * c:CH * (c + 1)], lg[:])

            arbufs = {}
            for k in range(STEPS):
                u_prev = fixedT if k == 0 else upds[(k + 1) % 2]
                u_cur = upds[k % 2]
                te_prev, te_cur = te[(k + 1) % 2], te[k % 2]
                ce_prev, ce_cur = ce[(k + 1) % 2], ce[k % 2]
                last = k == STEPS - 1

                def s1_pair(c0):
                    """S1 for chunks c0, c0+1 with same-weight matmuls
                    adjacent so LDWEIGHTS pre-loads and the second matmul
                    streams at full rate."""
                    sl0 = slice(CH * c0, CH * (c0 + 1))
                    sl1 = slice(CH * (c0 + 1), CH * (c0 + 2))
                    pa = pp1.tile([128, CH], F32, tag="p1",
                                  name=f"p1_{k}_{c0}")
                    pb = pp1.tile([128, CH], F32, tag="p1",
                                  name=f"p1_{k}_{c0 + 1}")
                    if k == 0:
                        nc.tensor.matmul(pa[:], wfu_t[:], fixedT[:, sl0],
                                         start=True, stop=False)
                        nc.tensor.matmul(pb[:], wfu_t[:], fixedT[:, sl1],
                                         start=True, stop=False)
                    else:
                        nc.tensor.matmul(pa[:], we1_fx, fixedT[:, sl0],
                                         start=True, stop=False)
                        nc.tensor.matmul(pb[:], we1_fx, fixedT[:, sl1],
                                         start=True, stop=False)
                        nc.tensor.matmul(pa[:], we1_up, u_prev[:, sl0],
                                         start=False, stop=False)
                        nc.tensor.matmul(pb[:], we1_up, u_prev[:, sl1],
                                         start=False, stop=False)
                    p1ring[c0] = pa
                    p1ring[c0 + 1] = pb

                p1ring = {}
                s1_pair(0)

                cls_state = {"nf": 0, "pend": []}

                def emit_cls(n):
                    if k == 0:
                        return
                    fronts = []
                    while n > 0 and cls_state["nf"] < NCH:
                        c = cls_state["nf"]
                        fronts.append((c, cls_front(u_prev, k - 1, c)))
                        cls_state["nf"] = c + 1
                        n -= 1
                    cls_state["pend"].extend(fronts)
                    while len(cls_state["pend"]) > 1:
                        c, hcp = cls_state["pend"].pop(0)
                        cls_back(k - 1, c, hcp)

                def finish_cls():
                    while cls_state["pend"]:
                        c, hcp = cls_state["pend"].pop(0)
                        cls_back(k - 1, c, hcp)

                # classifier cover: all 16 chunks for k==1 (the first AR
                # is cold and needs a much bigger shadow), else 6
                NCLS0 = NCH
                emit_cls(NCLS0)

                if k > 0:
                    # te(k) = mlp2([te(k-1), msga(k-1)])
                    pt1 = pp3.tile([128, AL], F32, tag="p3")
                    nc.tensor.matmul(pt1[:], wn1_a, te_prev[:],
                                     start=True, stop=False)
                    nc.tensor.matmul(pt1[:], wn1_b, msga[(k + 1) % 2][:],
                                     start=False, stop=True)
                    tn1 = wk.tile([128, AL], F32R, tag="tn1", bufs=2)
                    nc.scalar.activation(tn1[:], pt1[:], AF.Relu,
                                         bias=biases["bn1"][:, 0:1])
                    pt2 = pp3.tile([128, AL], F32, tag="p3")
                    nc.tensor.matmul(pt2[:], wn2_t[:], tn1[:],
                                     start=True, stop=True)
                    nc.scalar.activation(te_cur[:], pt2[:], AF.Relu,
                                         bias=biases["bn2"][:, 0:1])

                # Tb = W1a @ te(k) + be1  (h1 bias, (128, AL))
                pT = pp3.tile([128, AL], F32, tag="p3")
                nc.tensor.matmul(pT[:], we1_te, te_cur[:],
                                 start=True, stop=True)
                Tb = wk.tile([128, AL], F32, tag="tb", bufs=2)
                nc.scalar.activation(Tb[:], pT[:], AF.Identity,
                                     bias=biases["be1"][:, 0:1])

                # ===== AR(k-1)-dependent section =====
                # The ce-update chain alternates PE and scalar; classifier
                # chunks 6-8 are interleaved so the in-order PE queue has
                # independent work during the scalar hops.
                if k > 0:
                    mb_out_p, = arbufs.pop("out")
                    msgb_in = wk.tile([128, 2 * B], F32, tag="mbf", bufs=2)
                    nc.sync.dma_start(msgb_in[:, 0:B], mb_out_p[0:128, :])
                    nc.scalar.dma_start(msgb_in[:, B:2 * B],
                                        mb_out_p[128:256, :])
                    pc1 = pp3.tile([128, B], F32, tag="p3")
                    nc.tensor.matmul(pc1[:], wn1_a, ce_prev[:],
                                     start=True, stop=False)
                    nc.tensor.matmul(pc1[:], wn1_b,
                                     msgb_in[:, 0:B].bitcast(F32R),
                                     start=False, stop=False)
                    nc.tensor.matmul(pc1[:], wn1_b,
                                     msgb_in[:, B:2 * B].bitcast(F32R),
                                     start=False, stop=True)
                    cn1 = wk.tile([128, B], F32R, tag="cn1", bufs=2)
                    nc.scalar.activation(cn1[:], pc1[:], AF.Relu,
                                         bias=biases["bn1"][:, 0:1])
                    emit_cls(1)
                    pc2 = pp3.tile([128, B], F32, tag="p3")
                    nc.tensor.matmul(pc2[:], wn2_t[:], cn1[:],
                                     start=True, stop=True)
                    nc.scalar.activation(ce_cur[:], pc2[:], AF.Relu,
                                         bias=biases["bn2"][:, 0:1])
                    emit_cls(2)
                ce_b = ce_cur[:, :].to_broadcast((128, B, 2)) \
                    .transpose((0, 2, 1))

                accs = {}

                def do_pair(c):
                    g0 = u_cur[:, CH * c:CH * c + B].bitcast(F32)
                    g1 = u_cur[:, CH * c + B:CH * (c + 1)].bitcast(F32)
                    if c < 2:
                        acc = wk.tile([128, B], F32, tag="acc", bufs=4,
                                      name=f"acc{k}_{c}")
                        nc.vector.tensor_add(acc[:], g0, g1)
                        accs[c] = acc
                    else:
                        pr = wk.tile([128, B], F32, tag="pair", bufs=6,
                                     name=f"pr{k}_{c}")
                        nc.vector.tensor_add(pr[:], g0, g1)
                        nc.vector.tensor_add(accs[c % 2][:], accs[c % 2][:],
                                             pr[:])

                def msga_part(q):
                    # per-a sums over b for chunks 4q..4q+3 (contiguous)
                    seg = u_cur[:, 2048 * q:2048 * (q + 1)].bitcast(F32)
                    nc.vector.tensor_reduce(
                        msga_f[:, 8 * q:8 * (q + 1)],
                        seg.rearrange("p (a b) -> p a b", a=8),
                        mybir.AxisListType.X, ALU.add)

                if not last:
                    msga_f = wk.tile([128, AL], F32, tag="msgaf", bufs=2,
                                     name=f"msgaf{k}")

                for cp in range(0, NCH, 2):
                    p1s = [p1ring.pop(cp), p1ring.pop(cp + 1)]
                    for p1 in p1s:
                        nc.tensor.matmul(p1[:], we1_ce, ce_b,
                                         start=False, stop=True)
                    ts = []
                    for i, c in enumerate((cp, cp + 1)):
                        t = wk.tile([128, CH], F32R, tag="t", bufs=4,
                                    name=f"t{k}_{c}")
                        for g in range(2):
                            bsl = slice(B * g, B * (g + 1))
                            col = 2 * c + g
                            nc.scalar.activation(t[:, bsl], p1s[i][:, bsl],
                                                 AF.Relu,
                                                 bias=Tb[:, col:col + 1])
                        ts.append(t)
                    p2s = []
                    for t in ts:
                        p2 = pp2.tile([128, CH], F32, tag="p2")
                        nc.tensor.matmul(p2[:], we2_t[:], t[:],
                                         start=True, stop=True)
                        p2s.append(p2)
                    for i, c in enumerate((cp, cp + 1)):
                        sl = slice(CH * c, CH * (c + 1))
                        if last or c < 12:
                            nc.scalar.activation(u_cur[:, sl], p2s[i][:],
                                                 AF.Relu,
                                                 bias=biases["be2"][:, 0:1])
                        else:
                            # accum_out fills msga directly for the last 4
                            # chunks, keeping the step tail off the vector
                            # queue (the strided reduce there gated the AR
                            # trigger and the next step's te-update)
                            for g in range(2):
                                gsl = slice(CH * c + B * g,
                                            CH * c + B * (g + 1))
                                col = 2 * c + g
                                with nc.allow_low_precision(
                                        reason="f32r accum is 32-bit"):
                                    nc.scalar.activation(
                                        u_cur[:, gsl],
                                        p2s[i][:, B * g:B * (g + 1)],
                                        AF.Relu,
                                        bias=biases["be2"][:, 0:1],
                                        accum_out=msga[k % 2][:,
                                                           col:col + 1])
                    if cp + 2 < NCH:
                        s1_pair(cp + 2)
                    emit_cls(2)
                    if not last and cp >= 2:
                        do_pair(cp - 2)
                        do_pair(cp - 1)
                        if cp in (6, 10, 14):
                            msga_part(cp // 4 - 1)
                        if cp == 14:
                            nc.vector.tensor_copy(msga[k % 2][:, 0:24],
                                                  msga_f[:, 0:24])
                finish_cls()

                if not last:
                    do_pair(NCH - 2)
                    do_pair(NCH - 1)
                    # the two accumulator halves are summed by the CCE in
                    # the AllReduce itself (payload is latency-flat) and
                    # by the ce-update's accumulating matmuls afterwards
                    mb_in = dr.tile([2 * 128, B], F32, tag="mbin", bufs=2)
                    mb_out = dr.tile([2 * 128, B], F32, tag="mbout", bufs=2,
                                     addr_space="Shared")
                    nc.sync.dma_start(mb_in[0:128, :], accs[0][:])
                    nc.scalar.dma_start(mb_in[128:256, :], accs[1][:])
                    nc.gpsimd.collective_compute(
                        "AllReduce", mybir.AluOpType.add,
                        replica_groups=[list(range(8))],
                        ins=[mb_in.opt()], outs=[mb_out.opt()])
                    arbufs["out"] = (mb_out,)

            # final classifier for step 7
            u7 = upds[(STEPS - 1) % 2]
            hc_pend = None
            for c in range(NCH):
                hc_new = cls_front(u7, STEPS - 1, c)
                if hc_pend is not None:
                    cls_back(STEPS - 1, c - 1, hc_pend)
                hc_pend = hc_new
            cls_back(STEPS - 1, NCH - 1, hc_pend)

    nc.finalize()
    _BUILD_CACHE["nc"] = nc
    return nc


def _make_in_maps(inputs):
    f32 = np.float32

    def c(x):
        return np.ascontiguousarray(np.asarray(x, dtype=f32))

    tf = c(inputs["track_features"])
    cf = c(inputs["current_features"])
    tb = c(inputs["track_boxes"])
    cb = c(inputs["current_boxes"])
    tt = c(inputs["track_time"]).reshape(-1, 1)
    ct = c(inputs["current_time"]).reshape(-1, 1)

    shared = {
        "cfT": c(cf.T),
        "curf": cf,
        "curg": c(np.concatenate([cb, ct], axis=1)),
        "wlinT": c(inputs["w_lin"].T),
        "blin": c(np.broadcast_to(inputs["b_lin"][:, None], (D, 1))),
        "wein1T": c(inputs["w_ein1"].T),
        "bein1": c(inputs["b_ein1"][:, None]),
        "wein2T": c(inputs["w_ein2"].T),
        "bein2": c(inputs["b_ein2"][:, None]),
        "we1T": c(inputs["w_e1"].T),
        "be1": c(inputs["b_e1"][:, None]),
        "we2T": c(inputs["w_e2"].T),
        "be2": c(inputs["b_e2"][:, None]),
        "wn1T": c(inputs["w_n1"].T),
        "bn1": c(inputs["b_n1"][:, None]),
        "wn2T": c(inputs["w_n2"].T),
        "bn2": c(inputs["b_n2"][:, None]),
        "wc1T": c(inputs["w_c1"].T),
        "bc1": c(inputs["b_c1"][:, None]),
        "wc2c": c(inputs["w_c2"].T),
    }
    in_maps = []
    for core in range(8):
        rows = slice(AL * core, AL * (core + 1))
        m = dict(shared)
        m["tfT"] = c(tf[rows].T)
        m["trkf"] = c(tf[rows])
        m["trkg"] = c(np.concatenate([tb[rows], tt[rows]], axis=1))
        in_maps.append(m)
    return in_maps


def run(trace=False, trace_cores=None, **inputs):
    from concourse.bass_utils import run_bass_kernel_spmd

    if trace:
        _install_ntff_hook()
    nc = _build()
    in_maps = _make_in_maps(inputs)
    res = run_bass_kernel_spmd(nc, in_maps, core_ids=list(range(8)),
                               trace=trace, trace_cores=trace_cores)
    full = np.empty((STEPS, A, B), np.float32)
    for core in range(8):
        full[:, AL * core:AL * (core + 1), :] = \
            res.results[core]["out"].reshape(STEPS, AL, B)
    full += np.asarray(inputs["b_c2"], np.float32).reshape(1, 1, 1)
    return full, res


def kernel(**inputs):
    full, _ = run(trace=False, **inputs)
    return full


def _install_ntff_hook():
    import sys
    import types
    try:
        from antenv.axon_hooks import get_axon_ntff_profile_hook  # noqa: F401
        return
    except ImportError:
        pass
    import antenv
    from trn_agent_boot.trn_boot import _ntff_profile_via_ctypes

    mod = types.ModuleType("antenv.axon_hooks")
    holder = [_ntff_profile_via_ctypes("/opt/axon/libaxon_pjrt.so")]
    mod.get_axon_ntff_profile_hook = lambda: holder[0]
    mod.set_axon_ntff_profile_hook = lambda h: holder.__setitem__(0, h)
    sys.modules["antenv.axon_hooks"] = mod
    antenv.axon_hooks = mod
